# revision 30
# baseline (speedup 1.0000x reference)
"""DiffTransformerLayer on 8 trn2 NeuronCores.

Tensor-parallel attention: core c owns diff-head c (softmax heads 2c, 2c+1).
Per-sigma-block AllToAlls exchange per-head attention outputs; every core then
applies the full wo / FFN locally to its own 512 tokens.

Structure (vs the original version):
- LN1+transpose (A) and qkv projection (B) are interleaved per sigma block
  and share one PSUM tag FIFO, so B(s8) starts as soon as A(s8) is done.
- Softmax denominators: per-tau matmuls with constant ones / lam*ones
  [128,128] lhsT reduce over keys AND broadcast to all partitions in one
  stream (bz1 = lam*sum(e1) bcast, bz2 = sum(e2) bcast), replacing M=1
  z-matmuls + separate broadcast matmuls.
- The A2A ships the UNNORMALIZED oc (128 rows) plus its per-token sum of
  squares (row 129).  The RMS rsqrt runs post-A2A, batched per batch on a
  tiny [8,256] tile via Quake-style integer ops on the vector engine, so
  stage C issues no Sqrt/Ln on the scalar engine and the activation table
  never leaves the Exp set (no ACT_TABLE_LOAD thrash).
- Stage D(b) (wo matmuls) uses a dedicated PSUM tag so it overlaps batch
  b+1's attention instead of serializing behind it through pool-tag FIFOs.
- ln1_b / ln2_b are zero and subln_w uniform for this model's inputs; host
  detects that, folds (1-LAMBDA_INIT)*subln into wo, and drops bias ops.
"""

import sys

if "/opt/trn_rl_repo" not in sys.path:
    sys.path.insert(0, "/opt/trn_rl_repo")

import numpy as np

import concourse.bacc as bacc
import concourse.bass as bass
import concourse.tile as tile
from concourse import mybir
from concourse import bass_utils

F32 = mybir.dt.float32
F32R = mybir.dt.float32r
BF16 = mybir.dt.bfloat16
I32 = mybir.dt.int32
NP_BF16 = mybir.dt.np(BF16)

B, S, D = 2, 2048, 1024
H = 8
HD = 64
DEPTH = 12
LAMBDA_INIT = float(0.8 - 0.6 * np.exp(-0.3 * (DEPTH - 1)))
FFN = 2 * D
N_CORES = 8
NS = B * S                  # 4096 flattened tokens
DK = D // 128               # 8 feature tiles
NSIG = NS // 512            # 8 sigma blocks
NI = FFN // 128             # 16 inner-dim tiles
EPS = 1e-5
Exp = mybir.ActivationFunctionType.Exp
Ln = mybir.ActivationFunctionType.Ln
Silu = mybir.ActivationFunctionType.Silu
Ident = mybir.ActivationFunctionType.Identity
AluAdd = mybir.AluOpType.add
AluSub = mybir.AluOpType.subtract
AluMult = mybir.AluOpType.mult
AluShr = mybir.AluOpType.logical_shift_right
AluXor = mybir.AluOpType.bitwise_xor
RG = [list(range(N_CORES))]


def build_program(lam: float, zero_bias: bool, subln_c: float | None):
    """zero_bias: qkv/ffn biases (from ln*_b folding) are all zero.
    subln_c: if not None, subln_w*(1-LAMBDA_INIT) is uniform with this value
    (folded into wo on the host, so the device drops the subln multiply)."""
    nc = bacc.Bacc("TRN2", target_bir_lowering=False, debug=False,
                   enable_asserts=False, num_devices=N_CORES)

    x_nat = nc.dram_tensor("x_nat", [NS, D], BF16, kind="ExternalInput").ap()
    xT_own = nc.dram_tensor("xT_own", [D, 512], F32, kind="ExternalInput").ap()
    wq_s = nc.dram_tensor("wq_s", [D, 128], BF16, kind="ExternalInput").ap()
    wk_s = nc.dram_tensor("wk_s", [D, 128], BF16, kind="ExternalInput").ap()
    wv_s = nc.dram_tensor("wv_s", [D, 128], BF16, kind="ExternalInput").ap()
    wo_f = nc.dram_tensor("wo_f", [D, D], BF16, kind="ExternalInput").ap()
    w_in_f = nc.dram_tensor("w_in_f", [D, 2 * FFN], BF16, kind="ExternalInput").ap()
    w_out_f = nc.dram_tensor("w_out_f", [FFN, D], BF16, kind="ExternalInput").ap()
    masks_in = nc.dram_tensor("masks", [128, 4, 512], BF16, kind="ExternalInput").ap()
    sel_in = nc.dram_tensor("sel8", [8, 8, 128], BF16, kind="ExternalInput").ap()
    ident_in = nc.dram_tensor("ident", [128, 128], BF16, kind="ExternalInput").ap()
    if not zero_bias:
        qb_in = nc.dram_tensor("qb", [128], F32, kind="ExternalInput").ap()
        kb_in = nc.dram_tensor("kb", [128], F32, kind="ExternalInput").ap()
        vb4_in = nc.dram_tensor("vb4", [512], BF16, kind="ExternalInput").ap()
        inb_in = nc.dram_tensor("inb", [2 * FFN], F32, kind="ExternalInput").ap()
    if subln_c is None:
        subln_eff = nc.dram_tensor("subln_eff", [128], F32, kind="ExternalInput").ap()
    yT_out = nc.dram_tensor("yT", [D, 512], BF16, kind="ExternalOutput").ap()

    with tile.TileContext(nc) as tc:
        with (
            tc.tile_pool(name="persist", bufs=1) as persist,
            tc.tile_pool(name="ld", bufs=1) as ld,
            tc.tile_pool(name="stats", bufs=2) as stats,
            tc.tile_pool(name="dram", bufs=1, space="DRAM") as dram,
        ):
            # ---- constants ----
            ones_c = persist.tile([128, 1], BF16, tag="ones_c")
            nc.vector.memset(ones_c, 1.0)
            ones_mat = persist.tile([128, 128], BF16, tag="ones_mat")
            nc.vector.memset(ones_mat, 1.0)
            lam_mat = persist.tile([128, 128], BF16, tag="lam_mat")
            nc.vector.memset(lam_mat, float(lam))
            # one-hot selector rows (x sqrt(128)) for the post-A2A rstd
            # broadcast: rbc_h = sel[:,h,:].T @ rstd = sqrt(128)*rstd[h,:] bcast
            sel8 = persist.tile([8, 8, 128], BF16, tag="sel8")
            nc.sync.dma_start(out=sel8, in_=sel_in)
            rowinit = persist.tile([1, 128], F32, tag="rowinit")
            ones_rf = persist.tile([1, 128], F32R, tag="ones_rf")
            nc.vector.memset(rowinit, 1.0)
            with nc.allow_low_precision(reason="f32r constant rows"):
                nc.vector.tensor_copy(ones_rf, rowinit)
            eps128 = persist.tile([128, 1], F32, tag="eps128")
            nc.vector.memset(eps128, EPS)
            eps1 = persist.tile([1, 1], F32, tag="eps1")
            nc.vector.memset(eps1, EPS)
            # int32 scalar columns for the Quake rsqrt (AP scalars so the
            # bit patterns are exact; immediates lower as f32)
            qk_sh = persist.tile([8, 1], I32, tag="qk_sh")
            nc.vector.memset(qk_sh, 1)
            qk_m1 = persist.tile([8, 1], I32, tag="qk_m1")
            nc.vector.memset(qk_m1, -1)
            qk_mg = persist.tile([8, 256], I32, tag="qk_mg")
            nc.vector.memset(qk_mg, 0x5f3759e0)
            if subln_c is None:
                subln_t = persist.tile([128, 1], F32, tag="subln")
                nc.sync.dma_start(out=subln_t,
                                  in_=subln_eff.rearrange("(p one) -> p one", one=1))
            if not zero_bias:
                ones_rb = persist.tile([1, 128], BF16, tag="ones_rb")
                nc.vector.memset(ones_rb, 1.0)
                qb_t = persist.tile([128, 1], F32, tag="qb_t")
                nc.sync.dma_start(out=qb_t,
                                  in_=qb_in.rearrange("(p one) -> p one", one=1))
                kb_t = persist.tile([128, 1], F32, tag="kb_t")
                nc.sync.dma_start(out=kb_t,
                                  in_=kb_in.rearrange("(p one) -> p one", one=1))
                vb4_r = persist.tile([1, 512], BF16, tag="vb4_r")
                nc.sync.dma_start(out=vb4_r,
                                  in_=vb4_in.rearrange("(one f) -> one f", one=1))
                inb_t = persist.tile([128, 2 * NI], F32, tag="inb_t")
                nc.sync.dma_start(out=inb_t,
                                  in_=inb_in.rearrange("(k p) -> p k", p=128))

            # ---- persistent landing tiles ----
            pD_cm = tc.tile_pool(name="pD", bufs=1)
            pD = pD_cm.__enter__()
            wo2 = pD.tile([128, DK, D], BF16, tag="wo2")
            af = []
            for b in range(B):
                t = pD.tile([128, DK, 4, 64], BF16, tag=f"af{b}", name=f"af{b}")
                af.append(t)

            pqkv_cm = tc.tile_pool(name="pqkv", bufs=1)
            pqkv = pqkv_cm.__enter__()
            qT = [pqkv.tile([128, 512], BF16, tag=f"qT{s}", name=f"qT{s}")
                  for s in range(NSIG)]
            kT = [pqkv.tile([128, 512], BF16, tag=f"kT{s}", name=f"kT{s}")
                  for s in range(NSIG)]
            v_t = [pqkv.tile([128, 512], BF16, tag=f"v{s}", name=f"v{s}")
                   for s in range(NSIG)]
            ident = pqkv.tile([128, 128], BF16, tag="ident")
            masks = pqkv.tile([128, 4, 512], BF16, tag="masks")
            wq_sb = pqkv.tile([128, D], BF16, tag="wq_sb")
            wk_sb = pqkv.tile([128, D], BF16, tag="wk_sb")
            wv_sb = pqkv.tile([128, D], BF16, tag="wv_sb")

            # A2A payload per batch: 4 sigma blocks x (128 rows of
            # unnormalized oc + 1 row sum-of-squares)
            a2a_in = [dram.tile([N_CORES, 4, 129, 64], BF16, tag=f"a2ai{b}", name=f"a2ai{b}")
                      for b in range(B)]
            a2a_out = [dram.tile([N_CORES, 4, 129, 64], BF16, tag=f"a2ao{b}", name=f"a2ao{b}")
                       for b in range(B)]

            # single shared PSUM pool (8 banks):
            #   sc x3 (A-transposes + B-proj + C-scores + E-ffn, one FIFO),
            #   o1, o2, bz1, bz2 (C accumulators / E stats), pwo (D + E wout)
            psC_cm = tc.tile_pool(name="psC", bufs=1, space="PSUM")
            psC = psC_cm.__enter__()

            if not zero_bias:
                pbv = psC.tile([128, 512], F32, tag="o1")
                nc.tensor.matmul(pbv, lhsT=ones_rb, rhs=vb4_r, start=True, stop=True)
                bv_bc = pqkv.tile([128, 512], F32, tag="bv_bc")
                nc.vector.tensor_copy(bv_bc, pbv)

            # ========= Stage A+B interleaved: LN1 + transpose + qkv =========
            nc.sync.dma_start(out=ident, in_=ident_in)
            phT_cm = tc.tile_pool(name="phT", bufs=1)
            phT = phT_cm.__enter__()
            # hT double-buffered per sigma block (2 feature groups each)
            hTg = [[phT.tile([128, 4, 512], BF16, tag=f"hTg{g}_{sb}", name=f"hTg{g}_{sb}")
                    for sb in range(2)] for g in range(2)]
            for s8 in range(NSIG):
                x4 = []
                mvg = stats.tile([128, 4, 2], F32, tag="mvg")
                for j4 in range(4):
                    st = s8 * 4 + j4
                    x_t = ld.tile([128, D], BF16, tag="x_t", bufs=6)
                    nc.sync.dma_start(out=x_t, in_=x_nat[st * 128:(st + 1) * 128, :])
                    st_t = stats.tile([128, 2, 6], F32, tag="bst")
                    xg = x_t.rearrange("p (g d) -> p g d", g=2)
                    for g in range(2):
                        nc.vector.bn_stats(out=st_t[:, g, :], in_=xg[:, g, :])
                    nc.vector.bn_aggr(out=mvg[:, j4, :], in_=st_t)
                    x4.append(x_t)
                # rstd = 1/sqrt(var+eps); Sqrt is the only ACT table func in
                # stage A+B so the table loads exactly once here
                rt4 = stats.tile([128, 4], F32, tag="lnv")
                nc.scalar.activation(out=rt4, in_=mvg[:, :, 1],
                                     func=mybir.ActivationFunctionType.Sqrt,
                                     bias=eps128, scale=1.0)
                rstd4 = stats.tile([128, 4], F32, tag="rstd4")
                nc.vector.reciprocal(out=rstd4, in_=rt4)
                negmr4 = stats.tile([128, 4], F32, tag="negmr4")
                nc.vector.scalar_tensor_tensor(out=negmr4, in0=mvg[:, :, 0],
                                               scalar=-1.0, in1=rstd4,
                                               op0=AluMult, op1=AluMult)
                hT0 = hTg[0][s8 % 2]
                hT1 = hTg[1][s8 % 2]
                for j4 in range(4):
                    st = s8 * 4 + j4
                    h_t = ld.tile([128, D], BF16, tag="h_t", bufs=2)
                    if j4 % 2 == 0:
                        nc.scalar.activation(out=h_t, in_=x4[j4], func=Ident,
                                             scale=rstd4[:, j4:j4 + 1],
                                             bias=negmr4[:, j4:j4 + 1])
                    else:
                        nc.vector.tensor_scalar(out=h_t, in0=x4[j4],
                                                scalar1=mvg[:, j4, 0:1],
                                                scalar2=rstd4[:, j4:j4 + 1],
                                                op0=AluSub, op1=AluMult)
                    jcol = slice(j4 * 128, (j4 + 1) * 128)
                    for g4 in range(2):
                        tp = psC.tile([128, 512], BF16, tag="sc", bufs=4, name="tp")
                        for j in range(4):
                            dk = g4 * 4 + j
                            nc.tensor.transpose(tp[:, j * 128:(j + 1) * 128],
                                                h_t[:, dk * 128:(dk + 1) * 128], ident)
                        dst = (hT0 if g4 == 0 else hT1)[:, :, jcol]
                        srcv = tp.rearrange("p (j f) -> p j f", f=128)
                        if (st + g4) % 2 == 0:
                            nc.vector.tensor_copy(dst, srcv)
                        else:
                            nc.scalar.copy(dst, srcv)
                if s8 == 0:
                    # weight loads queued after the first token block so the
                    # LN1->qkv critical path gets the DMA queue first
                    nc.sync.dma_start(out=masks, in_=masks_in)
                    for sb_t, wsrc in ((wq_sb, wq_s), (wk_sb, wk_s), (wv_sb, wv_s)):
                        nc.sync.dma_start(
                            out=sb_t.rearrange("p (k m) -> p k m", m=128),
                            in_=wsrc.rearrange("(k p) m -> p k m", p=128))
                # ---- stage B for this sigma block ----
                sg = s8
                psq = psC.tile([128, 512], F32, tag="sc", bufs=4)
                for kk in range(DK):
                    nc.tensor.matmul(psq, lhsT=wq_sb[:, kk * 128:(kk + 1) * 128],
                                     rhs=(hT0 if kk < 4 else hT1)[:, kk % 4, :],
                                     start=(kk == 0), stop=(kk == DK - 1))
                if zero_bias:
                    nc.scalar.copy(qT[sg], psq)
                else:
                    nc.scalar.activation(out=qT[sg], in_=psq, func=Ident,
                                         scale=1.0, bias=qb_t)
                psk = psC.tile([128, 512], F32, tag="sc", bufs=4)
                for kk in range(DK):
                    nc.tensor.matmul(psk, lhsT=wk_sb[:, kk * 128:(kk + 1) * 128],
                                     rhs=(hT0 if kk < 4 else hT1)[:, kk % 4, :],
                                     start=(kk == 0), stop=(kk == DK - 1))
                if zero_bias:
                    nc.scalar.copy(kT[sg], psk)
                else:
                    nc.scalar.activation(out=kT[sg], in_=psk, func=Ident,
                                         scale=1.0, bias=kb_t)
                psv = psC.tile([128, 512], F32, tag="sc", bufs=4)
                for j4 in range(4):
                    for kk in range(DK):
                        nc.tensor.matmul(psv[:, j4 * 128:(j4 + 1) * 128],
                                         lhsT=(hT0 if kk < 4 else hT1)[:, kk % 4, j4 * 128:(j4 + 1) * 128],
                                         rhs=wv_sb[:, kk * 128:(kk + 1) * 128],
                                         start=(kk == 0), stop=(kk == DK - 1))
                if zero_bias:
                    nc.vector.tensor_copy(v_t[sg], psv)
                else:
                    nc.vector.tensor_add(v_t[sg], psv, bv_bc)
            phT_cm.__exit__(None, None, None)

            nc.sync.dma_start(out=wo2,
                              in_=wo_f.rearrange("(h p) m -> p h m", p=128))

            # ====== Stage C: differential attention (+ stage D interleaved) ======
            pwc_cm = tc.tile_pool(name="pwc", bufs=1)
            pwc = pwc_cm.__enter__()
            pE_cm = tc.tile_pool(name="pE", bufs=1)
            pE = pE_cm.__enter__()
            y1own = [persist.tile([128, 512], F32, tag=f"y1own{dk}", name=f"y1own{dk}")
                     for dk in range(DK)]
            y1bf = [persist.tile([128, 512], BF16, tag=f"y1bf{dk}", name=f"y1bf{dk}")
                    for dk in range(DK)]

            for b in range(B):
                for sl in range(4):
                    sg = 4 * b + sl
                    ntau = 4 * (sl + 1)
                    o1 = psC.tile([128, 512], F32, tag="o1")
                    o2 = psC.tile([128, 512], F32, tag="o2")
                    bz1 = psC.tile([128, 512], F32, tag="bz1")
                    bz2 = psC.tile([128, 512], F32, tag="bz2")
                    for tau in range(ntau):
                        tg = 16 * b + tau
                        ts8, tj = tg // 4, tg % 4
                        tcol = slice(tj * 128, (tj + 1) * 128)
                        rel = tau - 4 * sl
                        off = max(rel, 0) * 128          # causal column offset
                        ecol = slice(off, 512)
                        st_fl = (tau == 0)
                        sp_fl = (tau == ntau - 1)
                        s1 = psC.tile([128, 512], F32, tag="sc", bufs=4)
                        s2 = psC.tile([128, 512], F32, tag="sc", bufs=4)
                        nc.tensor.matmul(s1[:, ecol], lhsT=kT[ts8][0:64, tcol],
                                         rhs=qT[sg][0:64, ecol], start=True, stop=True)
                        nc.tensor.matmul(s2[:, ecol], lhsT=kT[ts8][64:128, tcol],
                                         rhs=qT[sg][64:128, ecol], start=True, stop=True)
                        e1 = pwc.tile([128, 512], BF16, tag="e1", bufs=4)
                        nc.scalar.activation(out=e1[:, ecol], in_=s1[:, ecol], func=Exp)
                        e2 = pwc.tile([128, 512], BF16, tag="e2", bufs=4)
                        nc.scalar.activation(out=e2[:, ecol], in_=s2[:, ecol], func=Exp)
                        if rel >= 0:
                            nc.gpsimd.tensor_mul(e1[:, ecol], e1[:, ecol],
                                                 masks[:, rel, ecol])
                            nc.vector.tensor_mul(e2[:, ecol], e2[:, ecol],
                                                 masks[:, rel, ecol])
                        nc.tensor.matmul(o1[:, ecol], lhsT=v_t[ts8][:, tcol],
                                         rhs=e1[:, ecol], start=st_fl, stop=sp_fl)
                        nc.tensor.matmul(bz1[:, ecol], lhsT=lam_mat,
                                         rhs=e1[:, ecol], start=st_fl, stop=sp_fl)
                        nc.tensor.matmul(o2[:, ecol], lhsT=v_t[ts8][:, tcol],
                                         rhs=e2[:, ecol], start=st_fl, stop=sp_fl)
                        nc.tensor.matmul(bz2[:, ecol], lhsT=ones_mat,
                                         rhs=e2[:, ecol], start=st_fl, stop=sp_fl)
                    # ---- combine: oc = o1 - (lam*z1/z2)*o2 (unnormalized).
                    # o1 is evacuated on ACT in parallel with the DVE chain so
                    # all four accumulator banks free early for the next
                    # iteration's matmuls. ----
                    o1sb = pwc.tile([128, 512], F32, tag="cw1", bufs=2)
                    nc.scalar.copy(o1sb, o1)
                    wden = pwc.tile([128, 512], F32, tag="cw2", bufs=2)
                    nc.vector.reciprocal_approx_fast(out=wden, in_=bz2)
                    w_sb = pwc.tile([128, 512], F32, tag="cw1", bufs=2)
                    nc.vector.tensor_mul(w_sb, bz1, wden)
                    t_sb = pwc.tile([128, 512], F32, tag="cw2", bufs=2)
                    nc.vector.tensor_mul(t_sb, o2, w_sb)
                    oc = pwc.tile([128, 512], BF16, tag="oc", bufs=2)
                    nc.vector.tensor_sub(oc, o1sb, t_sb)
                    sq = pwc.tile([128, 512], BF16, tag="sq", bufs=1)
                    nc.vector.tensor_mul(sq, oc, oc)
                    ssp = psC.tile([1, 512], F32, tag="sc", bufs=4)
                    nc.tensor.matmul(ssp, lhsT=ones_c, rhs=sq, start=True, stop=True)
                    ssr = pwc.tile([1, 512], BF16, tag="ssr", bufs=2)
                    nc.scalar.copy(ssr, ssp)
                    nc.sync.dma_start(
                        out=a2a_in[b][:, sl, 0:128, :].rearrange("u p f -> p u f"),
                        in_=oc.rearrange("p (u f) -> p u f", f=64))
                    nc.sync.dma_start(
                        out=a2a_in[b][:, sl, 128:129, :].rearrange("u one f -> one u f"),
                        in_=ssr.rearrange("one (u f) -> one u f", f=64))
                # one A2A for the whole batch (4 sigma blocks)
                nc.gpsimd.collective_compute(
                    "AllToAll", mybir.AluOpType.bypass, replica_groups=RG,
                    ins=[a2a_in[b].opt()], outs=[a2a_out[b].opt()])

            for b in range(B):
                # ---- stage D per batch. tile_wait_until pushes every D
                # instruction after all of stage C in the scheduler's engine
                # queues (sim-clock ordering only, no hardware waits), so
                # D's A2A-dependent work never head-of-line blocks C. ----
                tc.tile_set_cur_wait(10.0)
                nc.gpsimd.dma_start(
                    out=af[b].rearrange("p h a f -> p (h a) f"),
                    in_=a2a_out[b][:, :, 0:128, :].rearrange("h a p f -> p (h a) f"))
                ssb = stats.tile([8, 4, 64], BF16, tag="ssb", bufs=1)
                nc.gpsimd.dma_start(out=ssb,
                                    in_=a2a_out[b][:, :, 128, :])
                # rstd = rsqrt(ss) via Quake seed + 1 Newton step, all on DVE
                # (no ACT table funcs, so stage C's Exp table stays resident)
                ssv = ssb.rearrange("p a f -> p (a f)")          # [8, 256]
                ssf = stats.tile([8, 256], F32, tag="ssf", bufs=1)
                nc.vector.tensor_copy(ssf, ssv)
                y0 = stats.tile([8, 256], F32, tag="y0", bufs=1)
                nc.vector.tensor_scalar(out=y0.bitcast(I32), in0=ssf.bitcast(I32),
                                        scalar1=qk_sh, scalar2=qk_m1,
                                        op0=AluShr, op1=AluXor)
                nc.vector.tensor_add(y0.bitcast(I32), y0.bitcast(I32), qk_mg)
                t1 = stats.tile([8, 256], F32, tag="t1", bufs=1)
                nc.vector.tensor_mul(t1, y0, y0)
                nc.vector.tensor_mul(t1, t1, ssf)
                nc.vector.tensor_scalar(out=t1, in0=t1, scalar1=-0.5,
                                        scalar2=1.5, op0=AluMult, op1=AluAdd)
                rstd = stats.tile([8, 256], BF16, tag="rstd", bufs=1)
                nc.vector.tensor_mul(rstd, y0, t1)
                hcol = slice(b * 256, (b + 1) * 256)
                for h in range(DK):
                    # bcast rstd row h to 128 partitions (x sqrt(128)), then
                    # scale head h's oc in place
                    rbc = psC.tile([128, 256], F32, tag="sc", bufs=4, name="rbc")
                    nc.tensor.matmul(rbc, lhsT=sel8[:, h, :], rhs=rstd,
                                     start=True, stop=True)
                    nc.vector.tensor_mul(af[b][:, h, :, :],
                                         af[b][:, h, :, :],
                                         rbc.rearrange("p (a f) -> p a f", f=64))
                    if subln_c is None:
                        nc.vector.tensor_scalar_mul(af[b][:, h, :, :],
                                                    af[b][:, h, :, :], subln_t)
                for dm in range(DK):
                    xo_h = ld.tile([128, 256], F32, tag="xo_h", bufs=2)
                    nc.sync.dma_start(out=xo_h,
                                      in_=xT_own[dm * 128:(dm + 1) * 128, hcol])
                    pwo = psC.tile([128, 256], F32, tag="sc", bufs=4, name="pwo")
                    for h in range(DK):
                        nc.tensor.matmul(pwo,
                                         lhsT=wo2[:, h, dm * 128:(dm + 1) * 128],
                                         rhs=af[b][:, h, :, :], start=(h == 0),
                                         stop=(h == DK - 1))
                    nc.vector.tensor_add(y1own[dm][:, hcol], xo_h, pwo)
                    nc.scalar.copy(y1bf[dm][:, hcol], y1own[dm][:, hcol])

            tc.tile_set_cur_wait(12.0)
            # ================= Stage E: LN2 + FFN (local) =================
            ssum_t = psC.tile([128, 512], F32, tag="bz1")
            ssq_t = psC.tile([128, 512], F32, tag="bz2")
            ssum = ssum_t[0:1, :]
            ssq = ssq_t[0:1, :]
            for dk in range(DK):
                nc.tensor.matmul(ssum, lhsT=ones_c, rhs=y1bf[dk],
                                 start=(dk == 0), stop=(dk == DK - 1))
                sqt = ld.tile([128, 512], BF16, tag="sqt", bufs=1)
                nc.vector.tensor_mul(sqt, y1bf[dk], y1bf[dk])
                nc.tensor.matmul(ssq, lhsT=ones_c, rhs=sqt,
                                 start=(dk == 0), stop=(dk == DK - 1))
            m_row = stats.tile([1, 512], F32, tag="rowf1")
            nc.vector.tensor_scalar_mul(m_row, ssum, 1.0 / float(D))
            mm_row = stats.tile([1, 512], F32, tag="rowf2")
            nc.vector.tensor_mul(mm_row, m_row, m_row)
            v_row = stats.tile([1, 512], F32, tag="rowf3")
            nc.vector.tensor_scalar_mul(v_row, ssq, 1.0 / float(D))
            nc.vector.tensor_sub(v_row, v_row, mm_row)
            # r = 1/sqrt(var+eps)
            rtr = stats.tile([1, 512], F32, tag="rowf6")
            nc.scalar.activation(out=rtr, in_=v_row,
                                 func=mybir.ActivationFunctionType.Sqrt,
                                 bias=eps1, scale=1.0)
            rr = stats.tile([1, 512], F32, tag="rowf7")
            nc.vector.reciprocal(out=rr, in_=rtr)
            r_row = stats.tile([1, 512], F32R, tag="rowf4")
            mr_row = stats.tile([1, 512], F32R, tag="rowf5")
            with nc.allow_low_precision(reason="ln2 rows to f32r"):
                nc.vector.tensor_copy(r_row, rr)
                nc.vector.tensor_mul(mr_row, m_row, rr)
            pbc = psC.tile([128, 512], F32, tag="sc", bufs=4)
            nc.tensor.matmul(pbc, lhsT=ones_rf, rhs=r_row, start=True, stop=True)
            rbc2 = pE.tile([128, 512], BF16, tag="rbc2")
            nc.vector.tensor_copy(rbc2, pbc)
            pbc2 = psC.tile([128, 512], F32, tag="sc", bufs=4)
            nc.tensor.matmul(pbc2, lhsT=ones_rf, rhs=mr_row, start=True, stop=True)
            mrbc = pE.tile([128, 512], BF16, tag="mrbc")
            nc.vector.tensor_copy(mrbc, pbc2)
            h2 = []
            for dk in range(DK):
                a = pE.tile([128, 512], BF16, tag=f"h2{dk}", name=f"h2{dk}")
                eng = nc.vector if dk % 2 == 0 else nc.gpsimd
                eng.tensor_mul(a, y1bf[dk], rbc2)
                eng.tensor_sub(a, a, mrbc)
                h2.append(a)
            su = []
            for m in range(NI):
                wg = pE.tile([128, DK, 128], BF16, tag="wg", bufs=3)
                nc.sync.dma_start(
                    out=wg,
                    in_=w_in_f.rearrange("(k p) m -> p k m", p=128)[:, :, m * 128:(m + 1) * 128])
                wu = pE.tile([128, DK, 128], BF16, tag="wu", bufs=3)
                nc.sync.dma_start(
                    out=wu,
                    in_=w_in_f.rearrange("(k p) m -> p k m", p=128)[:, :, FFN + m * 128:FFN + (m + 1) * 128])
                psg = psC.tile([128, 512], F32, tag="sc", bufs=4)
                for kk in range(DK):
                    nc.tensor.matmul(psg, lhsT=wg[:, kk, :],
                                     rhs=h2[kk], start=(kk == 0), stop=(kk == DK - 1))
                psu = psC.tile([128, 512], F32, tag="sc", bufs=4)
                for kk in range(DK):
                    nc.tensor.matmul(psu, lhsT=wu[:, kk, :],
                                     rhs=h2[kk], start=(kk == 0), stop=(kk == DK - 1))
                sg_t = pE.tile([128, 512], BF16, tag="sg_t", bufs=1)
                su_t = pE.tile([128, 512], BF16, tag=f"su{m}", name=f"su{m}")
                if zero_bias:
                    nc.scalar.activation(out=sg_t, in_=psg, func=Silu, scale=1.0)
                    nc.vector.tensor_mul(su_t, psu, sg_t)
                else:
                    nc.scalar.activation(out=sg_t, in_=psg, func=Silu,
                                         scale=1.0, bias=inb_t[:, m:m + 1])
                    tu = pE.tile([128, 512], F32, tag="tu", bufs=2)
                    nc.vector.tensor_scalar_add(tu, psu, inb_t[:, NI + m:NI + m + 1])
                    nc.vector.tensor_mul(su_t, tu, sg_t)
                su.append(su_t)
            # ---- w_out + final residual, straight to output ----
            for dm in range(DK):
                wot = pE.tile([128, NI, 128], BF16, tag="wot", bufs=2)
                nc.sync.dma_start(
                    out=wot,
                    in_=w_out_f.rearrange("(k p) m -> p k m", p=128)[:, :, dm * 128:(dm + 1) * 128])
                py2 = psC.tile([128, 512], F32, tag="sc", bufs=4)
                for k in range(NI):
                    nc.tensor.matmul(py2, lhsT=wot[:, k, :],
                                     rhs=su[k], start=(k == 0), stop=(k == NI - 1))
                yout = ld.tile([128, 512], BF16, tag="yout", bufs=1)
                nc.vector.tensor_add(yout, y1own[dm], py2)
                nc.sync.dma_start(out=yT_out[dm * 128:(dm + 1) * 128, :], in_=yout)
            psC_cm.__exit__(None, None, None)
            pE_cm.__exit__(None, None, None)
            pwc_cm.__exit__(None, None, None)
            pqkv_cm.__exit__(None, None, None)
            pD_cm.__exit__(None, None, None)

    nc.compile()
    return nc


def _prep_inputs(inputs):
    """Host-side shard prep: returns (lam, zero_bias, subln_c, in_maps)."""
    f = {k: np.asarray(v, dtype=np.float32) for k, v in inputs.items()}
    lam = float(np.exp(np.sum(f["lq1"] * f["lk1"]))
                - np.exp(np.sum(f["lq2"] * f["lk2"])) + LAMBDA_INIT)
    x = f["x"].reshape(NS, D)
    x_bf = x.astype(NP_BF16)
    xT = np.ascontiguousarray(x.T)                       # [D, NS]
    pt = np.arange(128)[:, None, None]
    rl = np.arange(4)[None, :, None]
    cs = np.arange(512)[None, None, :]
    masks = (pt <= cs - 128 * rl).astype(NP_BF16)
    ident = np.eye(128, dtype=NP_BF16)
    subln_base = (f["subln_w"] * (1.0 - LAMBDA_INIT)).astype(np.float32)
    s8 = float(HD) ** -0.5
    l1w = f["ln1_w"][:, None]
    wq_e = l1w * f["wq"] * s8
    wk_e = l1w * f["wk"]
    wv_e = l1w * f["wv"]
    qb_full = f["ln1_b"] @ f["wq"] * s8                  # [D]
    kb_full = f["ln1_b"] @ f["wk"]
    vb_full = f["ln1_b"] @ f["wv"]
    w_in_e = (f["ln2_w"][:, None] * f["w_in"]).astype(NP_BF16)   # [D, 2*FFN]
    inb = (f["ln2_b"] @ f["w_in"]).astype(np.float32)            # [2*FFN]
    w_out_bf = f["w_out"].astype(NP_BF16)

    zero_bias = bool(np.all(qb_full == 0.0) and np.all(kb_full == 0.0)
                     and np.all(vb_full == 0.0) and np.all(inb == 0.0))
    subln_c = None
    if np.all(subln_base == subln_base[0]):
        subln_c = float(subln_base[0])
    wo_eff = f["wo"] * (subln_c if subln_c is not None else 1.0)
    wo_bf = wo_eff.astype(NP_BF16)

    sel8_np = (np.sqrt(128.0) * np.eye(8, dtype=np.float32)[:, :, None]
               * np.ones((1, 1, 128), np.float32)).astype(NP_BF16)
    in_maps = []
    for c in range(N_CORES):
        hc = slice(128 * c, 128 * (c + 1))
        xo = np.concatenate(
            [xT[:, b * S + 512 * sl + 64 * c: b * S + 512 * sl + 64 * c + 64]
             for b in range(B) for sl in range(4)], axis=1)
        m = {
            "x_nat": x_bf,
            "sel8": sel8_np,
            "xT_own": np.ascontiguousarray(xo),
            "wq_s": wq_e[:, hc].astype(NP_BF16),
            "wk_s": wk_e[:, hc].astype(NP_BF16),
            "wv_s": wv_e[:, hc].astype(NP_BF16),
            "wo_f": wo_bf,
            "w_in_f": w_in_e,
            "w_out_f": w_out_bf,
            "masks": masks, "ident": ident,
        }
        if not zero_bias:
            m["qb"] = np.ascontiguousarray(qb_full[hc])
            m["kb"] = np.ascontiguousarray(kb_full[hc])
            m["vb4"] = np.tile(vb_full[hc], 4).astype(NP_BF16)
            m["inb"] = inb
        if subln_c is None:
            m["subln_eff"] = subln_base
        in_maps.append(m)
    return lam, zero_bias, subln_c, in_maps


_CACHE = {}


def _run(inputs, trace=False, trace_kwargs=None):
    lam, zero_bias, subln_c, in_maps = _prep_inputs(inputs)
    key = (round(lam, 10), zero_bias, subln_c)
    if key not in _CACHE:
        _CACHE[key] = build_program(lam, zero_bias, subln_c)
    nc = _CACHE[key]
    res = bass_utils.run_bass_kernel_spmd(
        nc, in_maps, core_ids=list(range(N_CORES)),
        trace=trace, **(trace_kwargs or {}))
    y = np.empty((NS, D), dtype=np.float32)
    for c in range(N_CORES):
        yT = np.asarray(res.results[c]["yT"], dtype=np.float32)  # [D, 512]
        for b in range(B):
            for sl in range(4):
                fb = b * S + 512 * sl + 64 * c
                cb = (4 * b + sl) * 64
                y[fb:fb + 64, :] = yT[:, cb:cb + 64].T
    return y.reshape(B, S, D), res


def kernel(**inputs) -> np.ndarray:
    y, _ = _run(inputs)
    return y


# revision 31
# speedup vs baseline: 1.0039x; 1.0039x over previous
"""DiffTransformerLayer on 8 trn2 NeuronCores.

Tensor-parallel attention: core c owns diff-head c (softmax heads 2c, 2c+1).
Per-sigma-block AllToAlls exchange per-head attention outputs; every core then
applies the full wo / FFN locally to its own 512 tokens.

Structure (vs the original version):
- LN1+transpose (A) and qkv projection (B) are interleaved per sigma block
  and share one PSUM tag FIFO, so B(s8) starts as soon as A(s8) is done.
- Softmax denominators: per-tau matmuls with constant ones / lam*ones
  [128,128] lhsT reduce over keys AND broadcast to all partitions in one
  stream (bz1 = lam*sum(e1) bcast, bz2 = sum(e2) bcast), replacing M=1
  z-matmuls + separate broadcast matmuls.
- The A2A ships the UNNORMALIZED oc (128 rows) plus its per-token sum of
  squares (row 129).  The RMS rsqrt runs post-A2A, batched per batch on a
  tiny [8,256] tile via Quake-style integer ops on the vector engine, so
  stage C issues no Sqrt/Ln on the scalar engine and the activation table
  never leaves the Exp set (no ACT_TABLE_LOAD thrash).
- Stage D(b) (wo matmuls) uses a dedicated PSUM tag so it overlaps batch
  b+1's attention instead of serializing behind it through pool-tag FIFOs.
- ln1_b / ln2_b are zero and subln_w uniform for this model's inputs; host
  detects that, folds (1-LAMBDA_INIT)*subln into wo, and drops bias ops.
"""

import sys

if "/opt/trn_rl_repo" not in sys.path:
    sys.path.insert(0, "/opt/trn_rl_repo")

import numpy as np

import concourse.bacc as bacc
import concourse.bass as bass
import concourse.tile as tile
from concourse import mybir
from concourse import bass_utils

F32 = mybir.dt.float32
F32R = mybir.dt.float32r
BF16 = mybir.dt.bfloat16
I32 = mybir.dt.int32
NP_BF16 = mybir.dt.np(BF16)

B, S, D = 2, 2048, 1024
H = 8
HD = 64
DEPTH = 12
LAMBDA_INIT = float(0.8 - 0.6 * np.exp(-0.3 * (DEPTH - 1)))
FFN = 2 * D
N_CORES = 8
NS = B * S                  # 4096 flattened tokens
DK = D // 128               # 8 feature tiles
NSIG = NS // 512            # 8 sigma blocks
NI = FFN // 128             # 16 inner-dim tiles
EPS = 1e-5
Exp = mybir.ActivationFunctionType.Exp
Ln = mybir.ActivationFunctionType.Ln
Silu = mybir.ActivationFunctionType.Silu
Ident = mybir.ActivationFunctionType.Identity
AluAdd = mybir.AluOpType.add
AluSub = mybir.AluOpType.subtract
AluMult = mybir.AluOpType.mult
AluShr = mybir.AluOpType.logical_shift_right
AluXor = mybir.AluOpType.bitwise_xor
RG = [list(range(N_CORES))]


def build_program(lam: float, zero_bias: bool, subln_c: float | None):
    """zero_bias: qkv/ffn biases (from ln*_b folding) are all zero.
    subln_c: if not None, subln_w*(1-LAMBDA_INIT) is uniform with this value
    (folded into wo on the host, so the device drops the subln multiply)."""
    nc = bacc.Bacc("TRN2", target_bir_lowering=False, debug=False,
                   enable_asserts=False, num_devices=N_CORES)

    x_nat = nc.dram_tensor("x_nat", [NS, D], BF16, kind="ExternalInput").ap()
    xT_own = nc.dram_tensor("xT_own", [D, 512], F32, kind="ExternalInput").ap()
    wq_s = nc.dram_tensor("wq_s", [D, 128], BF16, kind="ExternalInput").ap()
    wk_s = nc.dram_tensor("wk_s", [D, 128], BF16, kind="ExternalInput").ap()
    wv_s = nc.dram_tensor("wv_s", [D, 128], BF16, kind="ExternalInput").ap()
    wo_f = nc.dram_tensor("wo_f", [D, D], BF16, kind="ExternalInput").ap()
    w_in_f = nc.dram_tensor("w_in_f", [D, 2 * FFN], BF16, kind="ExternalInput").ap()
    w_out_f = nc.dram_tensor("w_out_f", [FFN, D], BF16, kind="ExternalInput").ap()
    masks_in = nc.dram_tensor("masks", [128, 4, 512], BF16, kind="ExternalInput").ap()
    sel_in = nc.dram_tensor("sel8", [8, 8, 128], BF16, kind="ExternalInput").ap()
    ident_in = nc.dram_tensor("ident", [128, 128], BF16, kind="ExternalInput").ap()
    if not zero_bias:
        qb_in = nc.dram_tensor("qb", [128], F32, kind="ExternalInput").ap()
        kb_in = nc.dram_tensor("kb", [128], F32, kind="ExternalInput").ap()
        vb4_in = nc.dram_tensor("vb4", [512], BF16, kind="ExternalInput").ap()
        inb_in = nc.dram_tensor("inb", [2 * FFN], F32, kind="ExternalInput").ap()
    if subln_c is None:
        subln_eff = nc.dram_tensor("subln_eff", [128], F32, kind="ExternalInput").ap()
    yT_out = nc.dram_tensor("yT", [D, 512], BF16, kind="ExternalOutput").ap()

    with tile.TileContext(nc) as tc:
        with (
            tc.tile_pool(name="persist", bufs=1) as persist,
            tc.tile_pool(name="ld", bufs=1) as ld,
            tc.tile_pool(name="stats", bufs=2) as stats,
            tc.tile_pool(name="dram", bufs=1, space="DRAM") as dram,
        ):
            # ---- constants ----
            ones_c = persist.tile([128, 1], BF16, tag="ones_c")
            nc.vector.memset(ones_c, 1.0)
            ones_mat = persist.tile([128, 128], BF16, tag="ones_mat")
            nc.vector.memset(ones_mat, 1.0)
            lam_mat = persist.tile([128, 128], BF16, tag="lam_mat")
            nc.vector.memset(lam_mat, float(lam))
            # one-hot selector rows (x sqrt(128)) for the post-A2A rstd
            # broadcast: rbc_h = sel[:,h,:].T @ rstd = sqrt(128)*rstd[h,:] bcast
            sel8 = persist.tile([8, 8, 128], BF16, tag="sel8")
            nc.sync.dma_start(out=sel8, in_=sel_in)
            rowinit = persist.tile([1, 128], F32, tag="rowinit")
            ones_rf = persist.tile([1, 128], F32R, tag="ones_rf")
            nc.vector.memset(rowinit, 1.0)
            with nc.allow_low_precision(reason="f32r constant rows"):
                nc.vector.tensor_copy(ones_rf, rowinit)
            eps128 = persist.tile([128, 1], F32, tag="eps128")
            nc.vector.memset(eps128, EPS)
            eps1 = persist.tile([1, 1], F32, tag="eps1")
            nc.vector.memset(eps1, EPS)
            # int32 scalar columns for the Quake rsqrt (AP scalars so the
            # bit patterns are exact; immediates lower as f32)
            qk_sh = persist.tile([8, 1], I32, tag="qk_sh")
            nc.vector.memset(qk_sh, 1)
            qk_m1 = persist.tile([8, 1], I32, tag="qk_m1")
            nc.vector.memset(qk_m1, -1)
            qk_mg = persist.tile([8, 256], I32, tag="qk_mg")
            nc.vector.memset(qk_mg, 0x5f3759e0)
            if subln_c is None:
                subln_t = persist.tile([128, 1], F32, tag="subln")
                nc.sync.dma_start(out=subln_t,
                                  in_=subln_eff.rearrange("(p one) -> p one", one=1))
            if not zero_bias:
                ones_rb = persist.tile([1, 128], BF16, tag="ones_rb")
                nc.vector.memset(ones_rb, 1.0)
                qb_t = persist.tile([128, 1], F32, tag="qb_t")
                nc.sync.dma_start(out=qb_t,
                                  in_=qb_in.rearrange("(p one) -> p one", one=1))
                kb_t = persist.tile([128, 1], F32, tag="kb_t")
                nc.sync.dma_start(out=kb_t,
                                  in_=kb_in.rearrange("(p one) -> p one", one=1))
                vb4_r = persist.tile([1, 512], BF16, tag="vb4_r")
                nc.sync.dma_start(out=vb4_r,
                                  in_=vb4_in.rearrange("(one f) -> one f", one=1))
                inb_t = persist.tile([128, 2 * NI], F32, tag="inb_t")
                nc.sync.dma_start(out=inb_t,
                                  in_=inb_in.rearrange("(k p) -> p k", p=128))

            # ---- persistent landing tiles ----
            pD_cm = tc.tile_pool(name="pD", bufs=1)
            pD = pD_cm.__enter__()
            wo2 = pD.tile([128, DK, D], BF16, tag="wo2")
            af = []
            for b in range(B):
                t = pD.tile([128, DK, 4, 64], BF16, tag=f"af{b}", name=f"af{b}")
                af.append(t)

            pqkv_cm = tc.tile_pool(name="pqkv", bufs=1)
            pqkv = pqkv_cm.__enter__()
            qT = [pqkv.tile([128, 512], BF16, tag=f"qT{s}", name=f"qT{s}")
                  for s in range(NSIG)]
            kT = [pqkv.tile([128, 512], BF16, tag=f"kT{s}", name=f"kT{s}")
                  for s in range(NSIG)]
            v_t = [pqkv.tile([128, 512], BF16, tag=f"v{s}", name=f"v{s}")
                   for s in range(NSIG)]
            ident = pqkv.tile([128, 128], BF16, tag="ident")
            masks = pqkv.tile([128, 4, 512], BF16, tag="masks")
            wq_sb = pqkv.tile([128, D], BF16, tag="wq_sb")
            wk_sb = pqkv.tile([128, D], BF16, tag="wk_sb")
            wv_sb = pqkv.tile([128, D], BF16, tag="wv_sb")

            # A2A payload per batch: 4 sigma blocks x (128 rows of
            # unnormalized oc + 1 row sum-of-squares)
            a2a_in = [dram.tile([N_CORES, 4, 129, 64], BF16, tag=f"a2ai{b}", name=f"a2ai{b}")
                      for b in range(B)]
            a2a_out = [dram.tile([N_CORES, 4, 129, 64], BF16, tag=f"a2ao{b}", name=f"a2ao{b}")
                       for b in range(B)]

            # single shared PSUM pool (8 banks):
            #   sc x3 (A-transposes + B-proj + C-scores + E-ffn, one FIFO),
            #   o1, o2, bz1, bz2 (C accumulators / E stats), pwo (D + E wout)
            psC_cm = tc.tile_pool(name="psC", bufs=1, space="PSUM")
            psC = psC_cm.__enter__()

            if not zero_bias:
                pbv = psC.tile([128, 512], F32, tag="o1")
                nc.tensor.matmul(pbv, lhsT=ones_rb, rhs=vb4_r, start=True, stop=True)
                bv_bc = pqkv.tile([128, 512], F32, tag="bv_bc")
                nc.vector.tensor_copy(bv_bc, pbv)

            # ========= Stage A+B interleaved: LN1 + transpose + qkv =========
            nc.sync.dma_start(out=ident, in_=ident_in)
            phT_cm = tc.tile_pool(name="phT", bufs=1)
            phT = phT_cm.__enter__()
            # hT double-buffered per sigma block (2 feature groups each)
            hTg = [[phT.tile([128, 4, 512], BF16, tag=f"hTg{g}_{sb}", name=f"hTg{g}_{sb}")
                    for sb in range(2)] for g in range(2)]
            for s8 in range(NSIG):
                x4 = []
                mvg = stats.tile([128, 4, 2], F32, tag="mvg")
                for j4 in range(4):
                    st = s8 * 4 + j4
                    x_t = ld.tile([128, D], BF16, tag="x_t", bufs=5)
                    nc.sync.dma_start(out=x_t, in_=x_nat[st * 128:(st + 1) * 128, :])
                    st_t = stats.tile([128, 2, 6], F32, tag="bst")
                    xg = x_t.rearrange("p (g d) -> p g d", g=2)
                    for g in range(2):
                        nc.vector.bn_stats(out=st_t[:, g, :], in_=xg[:, g, :])
                    nc.vector.bn_aggr(out=mvg[:, j4, :], in_=st_t)
                    x4.append(x_t)
                # rstd = 1/sqrt(var+eps); Sqrt is the only ACT table func in
                # stage A+B so the table loads exactly once here
                rt4 = stats.tile([128, 4], F32, tag="lnv")
                nc.scalar.activation(out=rt4, in_=mvg[:, :, 1],
                                     func=mybir.ActivationFunctionType.Sqrt,
                                     bias=eps128, scale=1.0)
                rstd4 = stats.tile([128, 4], F32, tag="rstd4")
                nc.vector.reciprocal(out=rstd4, in_=rt4)
                negmr4 = stats.tile([128, 4], F32, tag="negmr4")
                nc.vector.scalar_tensor_tensor(out=negmr4, in0=mvg[:, :, 0],
                                               scalar=-1.0, in1=rstd4,
                                               op0=AluMult, op1=AluMult)
                hT0 = hTg[0][s8 % 2]
                hT1 = hTg[1][s8 % 2]
                for j4 in range(4):
                    st = s8 * 4 + j4
                    h_t = ld.tile([128, D], BF16, tag="h_t", bufs=3)
                    if j4 % 2 == 0:
                        nc.scalar.activation(out=h_t, in_=x4[j4], func=Ident,
                                             scale=rstd4[:, j4:j4 + 1],
                                             bias=negmr4[:, j4:j4 + 1])
                    else:
                        nc.vector.tensor_scalar(out=h_t, in0=x4[j4],
                                                scalar1=mvg[:, j4, 0:1],
                                                scalar2=rstd4[:, j4:j4 + 1],
                                                op0=AluSub, op1=AluMult)
                    jcol = slice(j4 * 128, (j4 + 1) * 128)
                    for g4 in range(2):
                        tp = psC.tile([128, 512], BF16, tag="sc", bufs=4, name="tp")
                        for j in range(4):
                            dk = g4 * 4 + j
                            nc.tensor.transpose(tp[:, j * 128:(j + 1) * 128],
                                                h_t[:, dk * 128:(dk + 1) * 128], ident)
                        dst = (hT0 if g4 == 0 else hT1)[:, :, jcol]
                        srcv = tp.rearrange("p (j f) -> p j f", f=128)
                        if (st + g4) % 2 == 0:
                            nc.vector.tensor_copy(dst, srcv)
                        else:
                            nc.scalar.copy(dst, srcv)
                if s8 == 0:
                    # weight loads queued after the first token block so the
                    # LN1->qkv critical path gets the DMA queue first
                    nc.sync.dma_start(out=masks, in_=masks_in)
                    for sb_t, wsrc in ((wq_sb, wq_s), (wk_sb, wk_s), (wv_sb, wv_s)):
                        nc.sync.dma_start(
                            out=sb_t.rearrange("p (k m) -> p k m", m=128),
                            in_=wsrc.rearrange("(k p) m -> p k m", p=128))
                # ---- stage B for this sigma block ----
                sg = s8
                psq = psC.tile([128, 512], F32, tag="sc", bufs=4)
                for kk in range(DK):
                    nc.tensor.matmul(psq, lhsT=wq_sb[:, kk * 128:(kk + 1) * 128],
                                     rhs=(hT0 if kk < 4 else hT1)[:, kk % 4, :],
                                     start=(kk == 0), stop=(kk == DK - 1))
                if zero_bias:
                    nc.scalar.copy(qT[sg], psq)
                else:
                    nc.scalar.activation(out=qT[sg], in_=psq, func=Ident,
                                         scale=1.0, bias=qb_t)
                psk = psC.tile([128, 512], F32, tag="sc", bufs=4)
                for kk in range(DK):
                    nc.tensor.matmul(psk, lhsT=wk_sb[:, kk * 128:(kk + 1) * 128],
                                     rhs=(hT0 if kk < 4 else hT1)[:, kk % 4, :],
                                     start=(kk == 0), stop=(kk == DK - 1))
                if zero_bias:
                    nc.scalar.copy(kT[sg], psk)
                else:
                    nc.scalar.activation(out=kT[sg], in_=psk, func=Ident,
                                         scale=1.0, bias=kb_t)
                psv = psC.tile([128, 512], F32, tag="sc", bufs=4)
                for j4 in range(4):
                    for kk in range(DK):
                        nc.tensor.matmul(psv[:, j4 * 128:(j4 + 1) * 128],
                                         lhsT=(hT0 if kk < 4 else hT1)[:, kk % 4, j4 * 128:(j4 + 1) * 128],
                                         rhs=wv_sb[:, kk * 128:(kk + 1) * 128],
                                         start=(kk == 0), stop=(kk == DK - 1))
                if zero_bias:
                    nc.vector.tensor_copy(v_t[sg], psv)
                else:
                    nc.vector.tensor_add(v_t[sg], psv, bv_bc)
            phT_cm.__exit__(None, None, None)

            nc.sync.dma_start(out=wo2,
                              in_=wo_f.rearrange("(h p) m -> p h m", p=128))

            # ====== Stage C: differential attention (+ stage D interleaved) ======
            pwc_cm = tc.tile_pool(name="pwc", bufs=1)
            pwc = pwc_cm.__enter__()
            pE_cm = tc.tile_pool(name="pE", bufs=1)
            pE = pE_cm.__enter__()
            y1own = [persist.tile([128, 512], F32, tag=f"y1own{dk}", name=f"y1own{dk}")
                     for dk in range(DK)]
            y1bf = [persist.tile([128, 512], BF16, tag=f"y1bf{dk}", name=f"y1bf{dk}")
                    for dk in range(DK)]

            for b in range(B):
                for sl in range(4):
                    sg = 4 * b + sl
                    ntau = 4 * (sl + 1)
                    o1 = psC.tile([128, 512], F32, tag="o1")
                    o2 = psC.tile([128, 512], F32, tag="o2")
                    bz1 = psC.tile([128, 512], F32, tag="bz1")
                    bz2 = psC.tile([128, 512], F32, tag="bz2")
                    for tau in range(ntau):
                        tg = 16 * b + tau
                        ts8, tj = tg // 4, tg % 4
                        tcol = slice(tj * 128, (tj + 1) * 128)
                        rel = tau - 4 * sl
                        off = max(rel, 0) * 128          # causal column offset
                        ecol = slice(off, 512)
                        st_fl = (tau == 0)
                        sp_fl = (tau == ntau - 1)
                        s1 = psC.tile([128, 512], F32, tag="sc", bufs=4)
                        s2 = psC.tile([128, 512], F32, tag="sc", bufs=4)
                        nc.tensor.matmul(s1[:, ecol], lhsT=kT[ts8][0:64, tcol],
                                         rhs=qT[sg][0:64, ecol], start=True, stop=True)
                        nc.tensor.matmul(s2[:, ecol], lhsT=kT[ts8][64:128, tcol],
                                         rhs=qT[sg][64:128, ecol], start=True, stop=True)
                        e1 = pwc.tile([128, 512], BF16, tag="e1", bufs=4)
                        nc.scalar.activation(out=e1[:, ecol], in_=s1[:, ecol], func=Exp)
                        e2 = pwc.tile([128, 512], BF16, tag="e2", bufs=4)
                        nc.scalar.activation(out=e2[:, ecol], in_=s2[:, ecol], func=Exp)
                        if rel >= 0:
                            nc.gpsimd.tensor_mul(e1[:, ecol], e1[:, ecol],
                                                 masks[:, rel, ecol])
                            nc.vector.tensor_mul(e2[:, ecol], e2[:, ecol],
                                                 masks[:, rel, ecol])
                        nc.tensor.matmul(o1[:, ecol], lhsT=v_t[ts8][:, tcol],
                                         rhs=e1[:, ecol], start=st_fl, stop=sp_fl)
                        nc.tensor.matmul(bz1[:, ecol], lhsT=lam_mat,
                                         rhs=e1[:, ecol], start=st_fl, stop=sp_fl)
                        nc.tensor.matmul(o2[:, ecol], lhsT=v_t[ts8][:, tcol],
                                         rhs=e2[:, ecol], start=st_fl, stop=sp_fl)
                        nc.tensor.matmul(bz2[:, ecol], lhsT=ones_mat,
                                         rhs=e2[:, ecol], start=st_fl, stop=sp_fl)
                    # ---- combine: oc = o1 - (lam*z1/z2)*o2 (unnormalized).
                    # o1 is evacuated on ACT in parallel with the DVE chain so
                    # all four accumulator banks free early for the next
                    # iteration's matmuls. ----
                    o1sb = pwc.tile([128, 512], F32, tag="cw1", bufs=2)
                    nc.scalar.copy(o1sb, o1)
                    wden = pwc.tile([128, 512], F32, tag="cw2", bufs=2)
                    nc.vector.reciprocal_approx_fast(out=wden, in_=bz2)
                    w_sb = pwc.tile([128, 512], F32, tag="cw1", bufs=2)
                    nc.vector.tensor_mul(w_sb, bz1, wden)
                    t_sb = pwc.tile([128, 512], F32, tag="cw2", bufs=2)
                    nc.vector.tensor_mul(t_sb, o2, w_sb)
                    oc = pwc.tile([128, 512], BF16, tag="oc", bufs=2)
                    nc.vector.tensor_sub(oc, o1sb, t_sb)
                    sq = pwc.tile([128, 512], BF16, tag="sq", bufs=1)
                    nc.vector.tensor_mul(sq, oc, oc)
                    ssp = psC.tile([1, 512], F32, tag="sc", bufs=4)
                    nc.tensor.matmul(ssp, lhsT=ones_c, rhs=sq, start=True, stop=True)
                    ssr = pwc.tile([1, 512], BF16, tag="ssr", bufs=2)
                    nc.scalar.copy(ssr, ssp)
                    nc.sync.dma_start(
                        out=a2a_in[b][:, sl, 0:128, :].rearrange("u p f -> p u f"),
                        in_=oc.rearrange("p (u f) -> p u f", f=64))
                    nc.sync.dma_start(
                        out=a2a_in[b][:, sl, 128:129, :].rearrange("u one f -> one u f"),
                        in_=ssr.rearrange("one (u f) -> one u f", f=64))
                # one A2A for the whole batch (4 sigma blocks)
                nc.gpsimd.collective_compute(
                    "AllToAll", mybir.AluOpType.bypass, replica_groups=RG,
                    ins=[a2a_in[b].opt()], outs=[a2a_out[b].opt()])

            for b in range(B):
                # ---- stage D per batch. tile_wait_until pushes every D
                # instruction after all of stage C in the scheduler's engine
                # queues (sim-clock ordering only, no hardware waits), so
                # D's A2A-dependent work never head-of-line blocks C. ----
                tc.tile_set_cur_wait(10.0)
                nc.gpsimd.dma_start(
                    out=af[b].rearrange("p h a f -> p (h a) f"),
                    in_=a2a_out[b][:, :, 0:128, :].rearrange("h a p f -> p (h a) f"))
                ssb = stats.tile([8, 4, 64], BF16, tag="ssb", bufs=1)
                nc.gpsimd.dma_start(out=ssb,
                                    in_=a2a_out[b][:, :, 128, :])
                # rstd = rsqrt(ss) via Quake seed + 1 Newton step, all on DVE
                # (no ACT table funcs, so stage C's Exp table stays resident)
                ssv = ssb.rearrange("p a f -> p (a f)")          # [8, 256]
                ssf = stats.tile([8, 256], F32, tag="ssf", bufs=1)
                nc.vector.tensor_copy(ssf, ssv)
                y0 = stats.tile([8, 256], F32, tag="y0", bufs=1)
                nc.vector.tensor_scalar(out=y0.bitcast(I32), in0=ssf.bitcast(I32),
                                        scalar1=qk_sh, scalar2=qk_m1,
                                        op0=AluShr, op1=AluXor)
                nc.vector.tensor_add(y0.bitcast(I32), y0.bitcast(I32), qk_mg)
                t1 = stats.tile([8, 256], F32, tag="t1", bufs=1)
                nc.vector.tensor_mul(t1, y0, y0)
                nc.vector.tensor_mul(t1, t1, ssf)
                nc.vector.tensor_scalar(out=t1, in0=t1, scalar1=-0.5,
                                        scalar2=1.5, op0=AluMult, op1=AluAdd)
                rstd = stats.tile([8, 256], BF16, tag="rstd", bufs=1)
                nc.vector.tensor_mul(rstd, y0, t1)
                hcol = slice(b * 256, (b + 1) * 256)
                for h in range(DK):
                    # bcast rstd row h to 128 partitions (x sqrt(128)), then
                    # scale head h's oc in place
                    rbc = psC.tile([128, 256], F32, tag="sc", bufs=4, name="rbc")
                    nc.tensor.matmul(rbc, lhsT=sel8[:, h, :], rhs=rstd,
                                     start=True, stop=True)
                    nc.vector.tensor_mul(af[b][:, h, :, :],
                                         af[b][:, h, :, :],
                                         rbc.rearrange("p (a f) -> p a f", f=64))
                    if subln_c is None:
                        nc.vector.tensor_scalar_mul(af[b][:, h, :, :],
                                                    af[b][:, h, :, :], subln_t)
                for dm in range(DK):
                    xo_h = ld.tile([128, 256], F32, tag="xo_h", bufs=2)
                    nc.sync.dma_start(out=xo_h,
                                      in_=xT_own[dm * 128:(dm + 1) * 128, hcol])
                    pwo = psC.tile([128, 256], F32, tag="sc", bufs=4, name="pwo")
                    for h in range(DK):
                        nc.tensor.matmul(pwo,
                                         lhsT=wo2[:, h, dm * 128:(dm + 1) * 128],
                                         rhs=af[b][:, h, :, :], start=(h == 0),
                                         stop=(h == DK - 1))
                    nc.vector.tensor_add(y1own[dm][:, hcol], xo_h, pwo)
                    nc.scalar.copy(y1bf[dm][:, hcol], y1own[dm][:, hcol])

            tc.tile_set_cur_wait(12.0)
            # ================= Stage E: LN2 + FFN (local) =================
            ssum_t = psC.tile([128, 512], F32, tag="bz1")
            ssq_t = psC.tile([128, 512], F32, tag="bz2")
            ssum = ssum_t[0:1, :]
            ssq = ssq_t[0:1, :]
            for dk in range(DK):
                nc.tensor.matmul(ssum, lhsT=ones_c, rhs=y1bf[dk],
                                 start=(dk == 0), stop=(dk == DK - 1))
                sqt = ld.tile([128, 512], BF16, tag="sqt", bufs=1)
                nc.vector.tensor_mul(sqt, y1bf[dk], y1bf[dk])
                nc.tensor.matmul(ssq, lhsT=ones_c, rhs=sqt,
                                 start=(dk == 0), stop=(dk == DK - 1))
            m_row = stats.tile([1, 512], F32, tag="rowf1")
            nc.vector.tensor_scalar_mul(m_row, ssum, 1.0 / float(D))
            mm_row = stats.tile([1, 512], F32, tag="rowf2")
            nc.vector.tensor_mul(mm_row, m_row, m_row)
            v_row = stats.tile([1, 512], F32, tag="rowf3")
            nc.vector.tensor_scalar_mul(v_row, ssq, 1.0 / float(D))
            nc.vector.tensor_sub(v_row, v_row, mm_row)
            # r = 1/sqrt(var+eps)
            rtr = stats.tile([1, 512], F32, tag="rowf6")
            nc.scalar.activation(out=rtr, in_=v_row,
                                 func=mybir.ActivationFunctionType.Sqrt,
                                 bias=eps1, scale=1.0)
            rr = stats.tile([1, 512], F32, tag="rowf7")
            nc.vector.reciprocal(out=rr, in_=rtr)
            r_row = stats.tile([1, 512], F32R, tag="rowf4")
            mr_row = stats.tile([1, 512], F32R, tag="rowf5")
            with nc.allow_low_precision(reason="ln2 rows to f32r"):
                nc.vector.tensor_copy(r_row, rr)
                nc.vector.tensor_mul(mr_row, m_row, rr)
            pbc = psC.tile([128, 512], F32, tag="sc", bufs=4)
            nc.tensor.matmul(pbc, lhsT=ones_rf, rhs=r_row, start=True, stop=True)
            rbc2 = pE.tile([128, 512], BF16, tag="rbc2")
            nc.vector.tensor_copy(rbc2, pbc)
            pbc2 = psC.tile([128, 512], F32, tag="sc", bufs=4)
            nc.tensor.matmul(pbc2, lhsT=ones_rf, rhs=mr_row, start=True, stop=True)
            mrbc = pE.tile([128, 512], BF16, tag="mrbc")
            nc.vector.tensor_copy(mrbc, pbc2)
            h2 = []
            for dk in range(DK):
                a = pE.tile([128, 512], BF16, tag=f"h2{dk}", name=f"h2{dk}")
                eng = nc.vector if dk % 2 == 0 else nc.gpsimd
                eng.tensor_mul(a, y1bf[dk], rbc2)
                eng.tensor_sub(a, a, mrbc)
                h2.append(a)
            su = []
            for m in range(NI):
                wg = pE.tile([128, DK, 128], BF16, tag="wg", bufs=3)
                nc.sync.dma_start(
                    out=wg,
                    in_=w_in_f.rearrange("(k p) m -> p k m", p=128)[:, :, m * 128:(m + 1) * 128])
                wu = pE.tile([128, DK, 128], BF16, tag="wu", bufs=3)
                nc.sync.dma_start(
                    out=wu,
                    in_=w_in_f.rearrange("(k p) m -> p k m", p=128)[:, :, FFN + m * 128:FFN + (m + 1) * 128])
                psg = psC.tile([128, 512], F32, tag="sc", bufs=4)
                for kk in range(DK):
                    nc.tensor.matmul(psg, lhsT=wg[:, kk, :],
                                     rhs=h2[kk], start=(kk == 0), stop=(kk == DK - 1))
                psu = psC.tile([128, 512], F32, tag="sc", bufs=4)
                for kk in range(DK):
                    nc.tensor.matmul(psu, lhsT=wu[:, kk, :],
                                     rhs=h2[kk], start=(kk == 0), stop=(kk == DK - 1))
                sg_t = pE.tile([128, 512], BF16, tag="sg_t", bufs=1)
                su_t = pE.tile([128, 512], BF16, tag=f"su{m}", name=f"su{m}")
                if zero_bias:
                    nc.scalar.activation(out=sg_t, in_=psg, func=Silu, scale=1.0)
                    nc.vector.tensor_mul(su_t, psu, sg_t)
                else:
                    nc.scalar.activation(out=sg_t, in_=psg, func=Silu,
                                         scale=1.0, bias=inb_t[:, m:m + 1])
                    tu = pE.tile([128, 512], F32, tag="tu", bufs=2)
                    nc.vector.tensor_scalar_add(tu, psu, inb_t[:, NI + m:NI + m + 1])
                    nc.vector.tensor_mul(su_t, tu, sg_t)
                su.append(su_t)
            # ---- w_out + final residual, straight to output ----
            for dm in range(DK):
                wot = pE.tile([128, NI, 128], BF16, tag="wot", bufs=2)
                nc.sync.dma_start(
                    out=wot,
                    in_=w_out_f.rearrange("(k p) m -> p k m", p=128)[:, :, dm * 128:(dm + 1) * 128])
                py2 = psC.tile([128, 512], F32, tag="sc", bufs=4)
                for k in range(NI):
                    nc.tensor.matmul(py2, lhsT=wot[:, k, :],
                                     rhs=su[k], start=(k == 0), stop=(k == NI - 1))
                yout = ld.tile([128, 512], BF16, tag="yout", bufs=1)
                nc.vector.tensor_add(yout, y1own[dm], py2)
                nc.sync.dma_start(out=yT_out[dm * 128:(dm + 1) * 128, :], in_=yout)
            psC_cm.__exit__(None, None, None)
            pE_cm.__exit__(None, None, None)
            pwc_cm.__exit__(None, None, None)
            pqkv_cm.__exit__(None, None, None)
            pD_cm.__exit__(None, None, None)

    nc.compile()
    return nc


def _prep_inputs(inputs):
    """Host-side shard prep: returns (lam, zero_bias, subln_c, in_maps)."""
    f = {k: np.asarray(v, dtype=np.float32) for k, v in inputs.items()}
    lam = float(np.exp(np.sum(f["lq1"] * f["lk1"]))
                - np.exp(np.sum(f["lq2"] * f["lk2"])) + LAMBDA_INIT)
    x = f["x"].reshape(NS, D)
    x_bf = x.astype(NP_BF16)
    xT = np.ascontiguousarray(x.T)                       # [D, NS]
    pt = np.arange(128)[:, None, None]
    rl = np.arange(4)[None, :, None]
    cs = np.arange(512)[None, None, :]
    masks = (pt <= cs - 128 * rl).astype(NP_BF16)
    ident = np.eye(128, dtype=NP_BF16)
    subln_base = (f["subln_w"] * (1.0 - LAMBDA_INIT)).astype(np.float32)
    s8 = float(HD) ** -0.5
    l1w = f["ln1_w"][:, None]
    wq_e = l1w * f["wq"] * s8
    wk_e = l1w * f["wk"]
    wv_e = l1w * f["wv"]
    qb_full = f["ln1_b"] @ f["wq"] * s8                  # [D]
    kb_full = f["ln1_b"] @ f["wk"]
    vb_full = f["ln1_b"] @ f["wv"]
    w_in_e = (f["ln2_w"][:, None] * f["w_in"]).astype(NP_BF16)   # [D, 2*FFN]
    inb = (f["ln2_b"] @ f["w_in"]).astype(np.float32)            # [2*FFN]
    w_out_bf = f["w_out"].astype(NP_BF16)

    zero_bias = bool(np.all(qb_full == 0.0) and np.all(kb_full == 0.0)
                     and np.all(vb_full == 0.0) and np.all(inb == 0.0))
    subln_c = None
    if np.all(subln_base == subln_base[0]):
        subln_c = float(subln_base[0])
    wo_eff = f["wo"] * (subln_c if subln_c is not None else 1.0)
    wo_bf = wo_eff.astype(NP_BF16)

    sel8_np = (np.sqrt(128.0) * np.eye(8, dtype=np.float32)[:, :, None]
               * np.ones((1, 1, 128), np.float32)).astype(NP_BF16)
    in_maps = []
    for c in range(N_CORES):
        hc = slice(128 * c, 128 * (c + 1))
        xo = np.concatenate(
            [xT[:, b * S + 512 * sl + 64 * c: b * S + 512 * sl + 64 * c + 64]
             for b in range(B) for sl in range(4)], axis=1)
        m = {
            "x_nat": x_bf,
            "sel8": sel8_np,
            "xT_own": np.ascontiguousarray(xo),
            "wq_s": wq_e[:, hc].astype(NP_BF16),
            "wk_s": wk_e[:, hc].astype(NP_BF16),
            "wv_s": wv_e[:, hc].astype(NP_BF16),
            "wo_f": wo_bf,
            "w_in_f": w_in_e,
            "w_out_f": w_out_bf,
            "masks": masks, "ident": ident,
        }
        if not zero_bias:
            m["qb"] = np.ascontiguousarray(qb_full[hc])
            m["kb"] = np.ascontiguousarray(kb_full[hc])
            m["vb4"] = np.tile(vb_full[hc], 4).astype(NP_BF16)
            m["inb"] = inb
        if subln_c is None:
            m["subln_eff"] = subln_base
        in_maps.append(m)
    return lam, zero_bias, subln_c, in_maps


_CACHE = {}


def _run(inputs, trace=False, trace_kwargs=None):
    lam, zero_bias, subln_c, in_maps = _prep_inputs(inputs)
    key = (round(lam, 10), zero_bias, subln_c)
    if key not in _CACHE:
        _CACHE[key] = build_program(lam, zero_bias, subln_c)
    nc = _CACHE[key]
    res = bass_utils.run_bass_kernel_spmd(
        nc, in_maps, core_ids=list(range(N_CORES)),
        trace=trace, **(trace_kwargs or {}))
    y = np.empty((NS, D), dtype=np.float32)
    for c in range(N_CORES):
        yT = np.asarray(res.results[c]["yT"], dtype=np.float32)  # [D, 512]
        for b in range(B):
            for sl in range(4):
                fb = b * S + 512 * sl + 64 * c
                cb = (4 * b + sl) * 64
                y[fb:fb + 64, :] = yT[:, cb:cb + 64].T
    return y.reshape(B, S, D), res


def kernel(**inputs) -> np.ndarray:
    y, _ = _run(inputs)
    return y


# revision 32
# speedup vs baseline: 1.1028x; 1.0985x over previous
"""DiffTransformerLayer on 8 trn2 NeuronCores.

Tensor-parallel attention: core c owns diff-head c (softmax heads 2c, 2c+1).
Per-sigma-block AllToAlls exchange per-head attention outputs; every core then
applies the full wo / FFN locally to its own 512 tokens.

Structure (vs the original version):
- LN1+transpose (A) and qkv projection (B) are interleaved per sigma block
  and share one PSUM tag FIFO, so B(s8) starts as soon as A(s8) is done.
- Softmax denominators: per-tau matmuls with constant ones / lam*ones
  [128,128] lhsT reduce over keys AND broadcast to all partitions in one
  stream (bz1 = lam*sum(e1) bcast, bz2 = sum(e2) bcast), replacing M=1
  z-matmuls + separate broadcast matmuls.
- The A2A ships the UNNORMALIZED oc (128 rows) plus its per-token sum of
  squares (row 129).  The RMS rsqrt runs post-A2A, batched per batch on a
  tiny [8,256] tile via Quake-style integer ops on the vector engine, so
  stage C issues no Sqrt/Ln on the scalar engine and the activation table
  never leaves the Exp set (no ACT_TABLE_LOAD thrash).
- Stage D(b) (wo matmuls) uses a dedicated PSUM tag so it overlaps batch
  b+1's attention instead of serializing behind it through pool-tag FIFOs.
- ln1_b / ln2_b are zero and subln_w uniform for this model's inputs; host
  detects that, folds (1-LAMBDA_INIT)*subln into wo, and drops bias ops.
"""

import sys

if "/opt/trn_rl_repo" not in sys.path:
    sys.path.insert(0, "/opt/trn_rl_repo")

import numpy as np

import concourse.bacc as bacc
import concourse.bass as bass
import concourse.tile as tile
from concourse import mybir
from concourse import bass_utils

F32 = mybir.dt.float32
F32R = mybir.dt.float32r
BF16 = mybir.dt.bfloat16
I32 = mybir.dt.int32
NP_BF16 = mybir.dt.np(BF16)

B, S, D = 2, 2048, 1024
H = 8
HD = 64
DEPTH = 12
LAMBDA_INIT = float(0.8 - 0.6 * np.exp(-0.3 * (DEPTH - 1)))
FFN = 2 * D
N_CORES = 8
NS = B * S                  # 4096 flattened tokens
DK = D // 128               # 8 feature tiles
NSIG = NS // 512            # 8 sigma blocks
NI = FFN // 128             # 16 inner-dim tiles
EPS = 1e-5
Exp = mybir.ActivationFunctionType.Exp
Ln = mybir.ActivationFunctionType.Ln
Silu = mybir.ActivationFunctionType.Silu
Ident = mybir.ActivationFunctionType.Identity
AluAdd = mybir.AluOpType.add
AluSub = mybir.AluOpType.subtract
AluMult = mybir.AluOpType.mult
AluShr = mybir.AluOpType.logical_shift_right
AluXor = mybir.AluOpType.bitwise_xor
RG = [list(range(N_CORES))]


def build_program(lam: float, zero_bias: bool, subln_c: float | None):
    """zero_bias: qkv/ffn biases (from ln*_b folding) are all zero.
    subln_c: if not None, subln_w*(1-LAMBDA_INIT) is uniform with this value
    (folded into wo on the host, so the device drops the subln multiply)."""
    nc = bacc.Bacc("TRN2", target_bir_lowering=False, debug=False,
                   enable_asserts=False, num_devices=N_CORES)

    x_nat = nc.dram_tensor("x_nat", [NS, D], BF16, kind="ExternalInput").ap()
    xT_own = nc.dram_tensor("xT_own", [D, 512], F32, kind="ExternalInput").ap()
    wq_s = nc.dram_tensor("wq_s", [D, 128], BF16, kind="ExternalInput").ap()
    wk_s = nc.dram_tensor("wk_s", [D, 128], BF16, kind="ExternalInput").ap()
    wv_s = nc.dram_tensor("wv_s", [D, 128], BF16, kind="ExternalInput").ap()
    wo_f = nc.dram_tensor("wo_f", [D, D], BF16, kind="ExternalInput").ap()
    w_in_f = nc.dram_tensor("w_in_f", [D, 2 * FFN], BF16, kind="ExternalInput").ap()
    w_out_f = nc.dram_tensor("w_out_f", [FFN, D], BF16, kind="ExternalInput").ap()
    masks_in = nc.dram_tensor("masks", [128, 4, 512], BF16, kind="ExternalInput").ap()
    sel_in = nc.dram_tensor("sel8", [8, 8, 128], BF16, kind="ExternalInput").ap()
    ident_in = nc.dram_tensor("ident", [128, 128], BF16, kind="ExternalInput").ap()
    if not zero_bias:
        qb_in = nc.dram_tensor("qb", [128], F32, kind="ExternalInput").ap()
        kb_in = nc.dram_tensor("kb", [128], F32, kind="ExternalInput").ap()
        vb4_in = nc.dram_tensor("vb4", [512], BF16, kind="ExternalInput").ap()
        inb_in = nc.dram_tensor("inb", [2 * FFN], F32, kind="ExternalInput").ap()
    if subln_c is None:
        subln_eff = nc.dram_tensor("subln_eff", [128], F32, kind="ExternalInput").ap()
    yT_out = nc.dram_tensor("yT", [D, 512], BF16, kind="ExternalOutput").ap()

    with tile.TileContext(nc) as tc:
        with (
            tc.tile_pool(name="persist", bufs=1) as persist,
            tc.tile_pool(name="ld", bufs=1) as ld,
            tc.tile_pool(name="stats", bufs=2) as stats,
            tc.tile_pool(name="dram", bufs=1, space="DRAM") as dram,
        ):
            # ---- constants ----
            ones_c = persist.tile([128, 1], BF16, tag="ones_c")
            nc.vector.memset(ones_c, 1.0)
            ones_mat = persist.tile([128, 128], BF16, tag="ones_mat")
            nc.vector.memset(ones_mat, 1.0)
            lam_mat = persist.tile([128, 128], BF16, tag="lam_mat")
            nc.vector.memset(lam_mat, float(lam))
            # one-hot selector rows (x sqrt(128)) for the post-A2A rstd
            # broadcast: rbc_h = sel[:,h,:].T @ rstd = sqrt(128)*rstd[h,:] bcast
            sel8 = persist.tile([8, 8, 128], BF16, tag="sel8")
            nc.sync.dma_start(out=sel8, in_=sel_in)
            rowinit = persist.tile([1, 128], F32, tag="rowinit")
            ones_rf = persist.tile([1, 128], F32R, tag="ones_rf")
            nc.vector.memset(rowinit, 1.0)
            with nc.allow_low_precision(reason="f32r constant rows"):
                nc.vector.tensor_copy(ones_rf, rowinit)
            eps128 = persist.tile([128, 1], F32, tag="eps128")
            nc.vector.memset(eps128, EPS)
            eps1 = persist.tile([1, 1], F32, tag="eps1")
            nc.vector.memset(eps1, EPS)
            # int32 scalar columns for the Quake rsqrt (AP scalars so the
            # bit patterns are exact; immediates lower as f32)
            qk_sh = persist.tile([8, 1], I32, tag="qk_sh")
            nc.vector.memset(qk_sh, 1)
            qk_m1 = persist.tile([8, 1], I32, tag="qk_m1")
            nc.vector.memset(qk_m1, -1)
            qk_mg = persist.tile([8, 256], I32, tag="qk_mg")
            nc.vector.memset(qk_mg, 0x5f3759e0)
            if subln_c is None:
                subln_t = persist.tile([128, 1], F32, tag="subln")
                nc.sync.dma_start(out=subln_t,
                                  in_=subln_eff.rearrange("(p one) -> p one", one=1))
            if not zero_bias:
                ones_rb = persist.tile([1, 128], BF16, tag="ones_rb")
                nc.vector.memset(ones_rb, 1.0)
                qb_t = persist.tile([128, 1], F32, tag="qb_t")
                nc.sync.dma_start(out=qb_t,
                                  in_=qb_in.rearrange("(p one) -> p one", one=1))
                kb_t = persist.tile([128, 1], F32, tag="kb_t")
                nc.sync.dma_start(out=kb_t,
                                  in_=kb_in.rearrange("(p one) -> p one", one=1))
                vb4_r = persist.tile([1, 512], BF16, tag="vb4_r")
                nc.sync.dma_start(out=vb4_r,
                                  in_=vb4_in.rearrange("(one f) -> one f", one=1))
                inb_t = persist.tile([128, 2 * NI], F32, tag="inb_t")
                nc.sync.dma_start(out=inb_t,
                                  in_=inb_in.rearrange("(k p) -> p k", p=128))

            # ---- persistent landing tiles ----
            pD_cm = tc.tile_pool(name="pD", bufs=1)
            pD = pD_cm.__enter__()
            wo2 = pD.tile([128, DK, D], BF16, tag="wo2")
            af = []
            for b in range(B):
                t = pD.tile([128, DK, 4, 64], BF16, tag=f"af{b}", name=f"af{b}")
                af.append(t)

            pqkv_cm = tc.tile_pool(name="pqkv", bufs=1)
            pqkv = pqkv_cm.__enter__()
            qT = [pqkv.tile([128, 512], BF16, tag=f"qT{s}", name=f"qT{s}")
                  for s in range(NSIG)]
            kT = [pqkv.tile([128, 512], BF16, tag=f"kT{s}", name=f"kT{s}")
                  for s in range(NSIG)]
            v_t = [pqkv.tile([128, 512], BF16, tag=f"v{s}", name=f"v{s}")
                   for s in range(NSIG)]
            ident = pqkv.tile([128, 128], BF16, tag="ident")
            masks = pqkv.tile([128, 4, 512], BF16, tag="masks")
            wq_sb = pqkv.tile([128, D], BF16, tag="wq_sb")
            wk_sb = pqkv.tile([128, D], BF16, tag="wk_sb")
            wv_sb = pqkv.tile([128, D], BF16, tag="wv_sb")

            # A2A payload per batch: 4 sigma blocks x (128 rows of
            # unnormalized oc + 1 row sum-of-squares)
            a2a_in = [dram.tile([N_CORES, 4, 129, 64], BF16, tag=f"a2ai{b}", name=f"a2ai{b}")
                      for b in range(B)]
            a2a_out = [dram.tile([N_CORES, 4, 129, 64], BF16, tag=f"a2ao{b}", name=f"a2ao{b}")
                       for b in range(B)]

            # single shared PSUM pool (8 banks):
            #   sc x3 (A-transposes + B-proj + C-scores + E-ffn, one FIFO),
            #   o1, o2, bz1, bz2 (C accumulators / E stats), pwo (D + E wout)
            psC_cm = tc.tile_pool(name="psC", bufs=1, space="PSUM")
            psC = psC_cm.__enter__()

            if not zero_bias:
                pbv = psC.tile([128, 512], F32, tag="o1")
                nc.tensor.matmul(pbv, lhsT=ones_rb, rhs=vb4_r, start=True, stop=True)
                bv_bc = pqkv.tile([128, 512], F32, tag="bv_bc")
                nc.vector.tensor_copy(bv_bc, pbv)

            # ========= Stage A+B interleaved: LN1 + transpose + qkv =========
            nc.sync.dma_start(out=ident, in_=ident_in)
            phT_cm = tc.tile_pool(name="phT", bufs=1)
            phT = phT_cm.__enter__()
            # hT double-buffered per sigma block (2 feature groups each)
            hTg = [[phT.tile([128, 4, 512], BF16, tag=f"hTg{g}_{sb}", name=f"hTg{g}_{sb}")
                    for sb in range(2)] for g in range(2)]
            for s8 in range(NSIG):
                x4 = []
                mvg = stats.tile([128, 4, 2], F32, tag="mvg")
                for j4 in range(4):
                    st = s8 * 4 + j4
                    x_t = ld.tile([128, D], BF16, tag="x_t", bufs=6)
                    nc.sync.dma_start(out=x_t, in_=x_nat[st * 128:(st + 1) * 128, :])
                    st_t = stats.tile([128, 2, 6], F32, tag="bst")
                    xg = x_t.rearrange("p (g d) -> p g d", g=2)
                    for g in range(2):
                        nc.vector.bn_stats(out=st_t[:, g, :], in_=xg[:, g, :])
                    nc.vector.bn_aggr(out=mvg[:, j4, :], in_=st_t)
                    x4.append(x_t)
                # rstd = 1/sqrt(var+eps); Sqrt is the only ACT table func in
                # stage A+B so the table loads exactly once here
                rt4 = stats.tile([128, 4], F32, tag="lnv")
                nc.scalar.activation(out=rt4, in_=mvg[:, :, 1],
                                     func=mybir.ActivationFunctionType.Sqrt,
                                     bias=eps128, scale=1.0)
                rstd4 = stats.tile([128, 4], F32, tag="rstd4")
                nc.vector.reciprocal(out=rstd4, in_=rt4)
                negmr4 = stats.tile([128, 4], F32, tag="negmr4")
                nc.vector.scalar_tensor_tensor(out=negmr4, in0=mvg[:, :, 0],
                                               scalar=-1.0, in1=rstd4,
                                               op0=AluMult, op1=AluMult)
                hT0 = hTg[0][s8 % 2]
                hT1 = hTg[1][s8 % 2]
                for j4 in range(4):
                    st = s8 * 4 + j4
                    h_t = ld.tile([128, D], BF16, tag="h_t", bufs=3)
                    if j4 % 2 == 0:
                        nc.scalar.activation(out=h_t, in_=x4[j4], func=Ident,
                                             scale=rstd4[:, j4:j4 + 1],
                                             bias=negmr4[:, j4:j4 + 1])
                    else:
                        nc.vector.tensor_scalar(out=h_t, in0=x4[j4],
                                                scalar1=mvg[:, j4, 0:1],
                                                scalar2=rstd4[:, j4:j4 + 1],
                                                op0=AluSub, op1=AluMult)
                    jcol = slice(j4 * 128, (j4 + 1) * 128)
                    for g4 in range(2):
                        tp = psC.tile([128, 512], BF16, tag="sc", bufs=4, name="tp")
                        for j in range(4):
                            dk = g4 * 4 + j
                            nc.tensor.transpose(tp[:, j * 128:(j + 1) * 128],
                                                h_t[:, dk * 128:(dk + 1) * 128], ident)
                        dst = (hT0 if g4 == 0 else hT1)[:, :, jcol]
                        srcv = tp.rearrange("p (j f) -> p j f", f=128)
                        if (st + g4) % 2 == 0:
                            nc.vector.tensor_copy(dst, srcv)
                        else:
                            nc.scalar.copy(dst, srcv)
                if s8 == 0:
                    # weight loads queued after the first token block so the
                    # LN1->qkv critical path gets the DMA queue first
                    nc.sync.dma_start(out=masks, in_=masks_in)
                    for sb_t, wsrc in ((wq_sb, wq_s), (wk_sb, wk_s), (wv_sb, wv_s)):
                        nc.sync.dma_start(
                            out=sb_t.rearrange("p (k m) -> p k m", m=128),
                            in_=wsrc.rearrange("(k p) m -> p k m", p=128))
                # ---- stage B for this sigma block ----
                sg = s8
                psq = psC.tile([128, 512], F32, tag="sc", bufs=4)
                for kk in range(DK):
                    nc.tensor.matmul(psq, lhsT=wq_sb[:, kk * 128:(kk + 1) * 128],
                                     rhs=(hT0 if kk < 4 else hT1)[:, kk % 4, :],
                                     start=(kk == 0), stop=(kk == DK - 1))
                if zero_bias:
                    nc.scalar.copy(qT[sg], psq)
                else:
                    nc.scalar.activation(out=qT[sg], in_=psq, func=Ident,
                                         scale=1.0, bias=qb_t)
                psk = psC.tile([128, 512], F32, tag="sc", bufs=4)
                for kk in range(DK):
                    nc.tensor.matmul(psk, lhsT=wk_sb[:, kk * 128:(kk + 1) * 128],
                                     rhs=(hT0 if kk < 4 else hT1)[:, kk % 4, :],
                                     start=(kk == 0), stop=(kk == DK - 1))
                if zero_bias:
                    nc.scalar.copy(kT[sg], psk)
                else:
                    nc.scalar.activation(out=kT[sg], in_=psk, func=Ident,
                                         scale=1.0, bias=kb_t)
                psv = psC.tile([128, 512], F32, tag="sc", bufs=4)
                for j4 in range(4):
                    for kk in range(DK):
                        nc.tensor.matmul(psv[:, j4 * 128:(j4 + 1) * 128],
                                         lhsT=(hT0 if kk < 4 else hT1)[:, kk % 4, j4 * 128:(j4 + 1) * 128],
                                         rhs=wv_sb[:, kk * 128:(kk + 1) * 128],
                                         start=(kk == 0), stop=(kk == DK - 1))
                if zero_bias:
                    nc.vector.tensor_copy(v_t[sg], psv)
                else:
                    nc.vector.tensor_add(v_t[sg], psv, bv_bc)
            phT_cm.__exit__(None, None, None)

            nc.sync.dma_start(out=wo2,
                              in_=wo_f.rearrange("(h p) m -> p h m", p=128))

            # ====== Stage C: differential attention (+ stage D interleaved) ======
            pwc_cm = tc.tile_pool(name="pwc", bufs=1)
            pwc = pwc_cm.__enter__()
            pE_cm = tc.tile_pool(name="pE", bufs=1)
            pE = pE_cm.__enter__()
            y1own = [persist.tile([128, 512], F32, tag=f"y1own{dk}", name=f"y1own{dk}")
                     for dk in range(DK)]
            y1bf = [persist.tile([128, 512], BF16, tag=f"y1bf{dk}", name=f"y1bf{dk}")
                    for dk in range(DK)]

            for b in range(B):
                for sl in range(4):
                    sg = 4 * b + sl
                    ntau = 4 * (sl + 1)
                    o1 = psC.tile([128, 512], F32, tag="o1")
                    o2 = psC.tile([128, 512], F32, tag="o2")
                    bz1 = psC.tile([128, 512], F32, tag="bz1")
                    bz2 = psC.tile([128, 512], F32, tag="bz2")
                    for tau in range(ntau):
                        tg = 16 * b + tau
                        ts8, tj = tg // 4, tg % 4
                        tcol = slice(tj * 128, (tj + 1) * 128)
                        rel = tau - 4 * sl
                        off = max(rel, 0) * 128          # causal column offset
                        ecol = slice(off, 512)
                        st_fl = (tau == 0)
                        sp_fl = (tau == ntau - 1)
                        s1 = psC.tile([128, 512], F32, tag="sc", bufs=4)
                        s2 = psC.tile([128, 512], F32, tag="sc", bufs=4)
                        nc.tensor.matmul(s1[:, ecol], lhsT=kT[ts8][0:64, tcol],
                                         rhs=qT[sg][0:64, ecol], start=True, stop=True)
                        nc.tensor.matmul(s2[:, ecol], lhsT=kT[ts8][64:128, tcol],
                                         rhs=qT[sg][64:128, ecol], start=True, stop=True)
                        e1 = pwc.tile([128, 512], BF16, tag="e1", bufs=4)
                        nc.scalar.activation(out=e1[:, ecol], in_=s1[:, ecol], func=Exp)
                        e2 = pwc.tile([128, 512], BF16, tag="e2", bufs=4)
                        nc.scalar.activation(out=e2[:, ecol], in_=s2[:, ecol], func=Exp)
                        if rel >= 0:
                            nc.gpsimd.tensor_mul(e1[:, ecol], e1[:, ecol],
                                                 masks[:, rel, ecol])
                            nc.gpsimd.tensor_mul(e2[:, ecol], e2[:, ecol],
                                                 masks[:, rel, ecol])
                        nc.tensor.matmul(o1[:, ecol], lhsT=v_t[ts8][:, tcol],
                                         rhs=e1[:, ecol], start=st_fl, stop=sp_fl)
                        nc.tensor.matmul(bz1[:, ecol], lhsT=lam_mat,
                                         rhs=e1[:, ecol], start=st_fl, stop=sp_fl)
                        nc.tensor.matmul(o2[:, ecol], lhsT=v_t[ts8][:, tcol],
                                         rhs=e2[:, ecol], start=st_fl, stop=sp_fl)
                        nc.tensor.matmul(bz2[:, ecol], lhsT=ones_mat,
                                         rhs=e2[:, ecol], start=st_fl, stop=sp_fl)
                    # ---- combine: oc = o1 - (lam*z1/z2)*o2 (unnormalized) ----
                    wden = pwc.tile([128, 512], F32, tag="t_a", bufs=1)
                    nc.vector.reciprocal_approx_fast(out=wden, in_=bz2)
                    w_sb = pwc.tile([128, 512], F32, tag="t_b", bufs=1)
                    nc.vector.tensor_mul(w_sb, bz1, wden)
                    t_sb = pwc.tile([128, 512], F32, tag="t_a", bufs=1)
                    nc.vector.tensor_mul(t_sb, o2, w_sb)
                    oc = pwc.tile([128, 512], BF16, tag="oc", bufs=2)
                    nc.vector.tensor_sub(oc, o1, t_sb)
                    sq = pwc.tile([128, 512], BF16, tag="sq", bufs=1)
                    nc.vector.tensor_mul(sq, oc, oc)
                    ssp = psC.tile([1, 512], F32, tag="sc", bufs=4)
                    nc.tensor.matmul(ssp, lhsT=ones_c, rhs=sq, start=True, stop=True)
                    ssr = pwc.tile([1, 512], BF16, tag="ssr", bufs=2)
                    nc.scalar.copy(ssr, ssp)
                    nc.sync.dma_start(
                        out=a2a_in[b][:, sl, 0:128, :].rearrange("u p f -> p u f"),
                        in_=oc.rearrange("p (u f) -> p u f", f=64))
                    nc.sync.dma_start(
                        out=a2a_in[b][:, sl, 128:129, :].rearrange("u one f -> one u f"),
                        in_=ssr.rearrange("one (u f) -> one u f", f=64))
                # one A2A for the whole batch (4 sigma blocks)
                nc.gpsimd.collective_compute(
                    "AllToAll", mybir.AluOpType.bypass, replica_groups=RG,
                    ins=[a2a_in[b].opt()], outs=[a2a_out[b].opt()])

            for b in range(B):
                # ---- stage D per batch. tile_wait_until pushes every D
                # instruction after all of stage C in the scheduler's engine
                # queues (sim-clock ordering only, no hardware waits), so
                # D's A2A-dependent work never head-of-line blocks C. ----
                tc.tile_set_cur_wait(10.0 + b)
                nc.gpsimd.dma_start(
                    out=af[b].rearrange("p h a f -> p (h a) f"),
                    in_=a2a_out[b][:, :, 0:128, :].rearrange("h a p f -> p (h a) f"))
                ssb = stats.tile([8, 4, 64], BF16, tag="ssb", bufs=1)
                nc.gpsimd.dma_start(out=ssb,
                                    in_=a2a_out[b][:, :, 128, :])
                # rstd = rsqrt(ss) via Quake seed + 1 Newton step, all on DVE
                # (no ACT table funcs, so stage C's Exp table stays resident)
                ssv = ssb.rearrange("p a f -> p (a f)")          # [8, 256]
                ssf = stats.tile([8, 256], F32, tag="ssf", bufs=1)
                nc.vector.tensor_copy(ssf, ssv)
                y0 = stats.tile([8, 256], F32, tag="y0", bufs=1)
                nc.vector.tensor_scalar(out=y0.bitcast(I32), in0=ssf.bitcast(I32),
                                        scalar1=qk_sh, scalar2=qk_m1,
                                        op0=AluShr, op1=AluXor)
                nc.vector.tensor_add(y0.bitcast(I32), y0.bitcast(I32), qk_mg)
                t1 = stats.tile([8, 256], F32, tag="t1", bufs=1)
                nc.vector.tensor_mul(t1, y0, y0)
                nc.vector.tensor_mul(t1, t1, ssf)
                nc.vector.tensor_scalar(out=t1, in0=t1, scalar1=-0.5,
                                        scalar2=1.5, op0=AluMult, op1=AluAdd)
                rstd = stats.tile([8, 256], BF16, tag="rstd", bufs=1)
                nc.vector.tensor_mul(rstd, y0, t1)
                hcol = slice(b * 256, (b + 1) * 256)
                for h in range(DK):
                    # bcast rstd row h to 128 partitions (x sqrt(128)), then
                    # scale head h's oc in place
                    rbc = psC.tile([128, 256], F32, tag="sc", bufs=4, name="rbc")
                    nc.tensor.matmul(rbc, lhsT=sel8[:, h, :], rhs=rstd,
                                     start=True, stop=True)
                    nc.vector.tensor_mul(af[b][:, h, :, :],
                                         af[b][:, h, :, :],
                                         rbc.rearrange("p (a f) -> p a f", f=64))
                    if subln_c is None:
                        nc.vector.tensor_scalar_mul(af[b][:, h, :, :],
                                                    af[b][:, h, :, :], subln_t)
                for dm in range(DK):
                    xo_h = ld.tile([128, 256], F32, tag="xo_h", bufs=3)
                    nc.sync.dma_start(out=xo_h,
                                      in_=xT_own[dm * 128:(dm + 1) * 128, hcol])
                    pwo = psC.tile([128, 256], F32, tag="sc", bufs=4, name="pwo")
                    for h in range(DK):
                        nc.tensor.matmul(pwo,
                                         lhsT=wo2[:, h, dm * 128:(dm + 1) * 128],
                                         rhs=af[b][:, h, :, :], start=(h == 0),
                                         stop=(h == DK - 1))
                    nc.vector.tensor_add(y1own[dm][:, hcol], xo_h, pwo)
                    nc.scalar.copy(y1bf[dm][:, hcol], y1own[dm][:, hcol])

            tc.tile_set_cur_wait(12.0)
            # ================= Stage E: LN2 + FFN (local) =================
            ssum_t = psC.tile([128, 512], F32, tag="bz1")
            ssq_t = psC.tile([128, 512], F32, tag="bz2")
            ssum = ssum_t[0:1, :]
            ssq = ssq_t[0:1, :]
            for dk in range(DK):
                nc.tensor.matmul(ssum, lhsT=ones_c, rhs=y1bf[dk],
                                 start=(dk == 0), stop=(dk == DK - 1))
                sqt = ld.tile([128, 512], BF16, tag="sqt", bufs=1)
                nc.vector.tensor_mul(sqt, y1bf[dk], y1bf[dk])
                nc.tensor.matmul(ssq, lhsT=ones_c, rhs=sqt,
                                 start=(dk == 0), stop=(dk == DK - 1))
            m_row = stats.tile([1, 512], F32, tag="rowf1")
            nc.vector.tensor_scalar_mul(m_row, ssum, 1.0 / float(D))
            mm_row = stats.tile([1, 512], F32, tag="rowf2")
            nc.vector.tensor_mul(mm_row, m_row, m_row)
            v_row = stats.tile([1, 512], F32, tag="rowf3")
            nc.vector.tensor_scalar_mul(v_row, ssq, 1.0 / float(D))
            nc.vector.tensor_sub(v_row, v_row, mm_row)
            # r = 1/sqrt(var+eps)
            rtr = stats.tile([1, 512], F32, tag="rowf6")
            nc.scalar.activation(out=rtr, in_=v_row,
                                 func=mybir.ActivationFunctionType.Sqrt,
                                 bias=eps1, scale=1.0)
            rr = stats.tile([1, 512], F32, tag="rowf7")
            nc.vector.reciprocal(out=rr, in_=rtr)
            r_row = stats.tile([1, 512], F32R, tag="rowf4")
            mr_row = stats.tile([1, 512], F32R, tag="rowf5")
            with nc.allow_low_precision(reason="ln2 rows to f32r"):
                nc.vector.tensor_copy(r_row, rr)
                nc.vector.tensor_mul(mr_row, m_row, rr)
            pbc = psC.tile([128, 512], F32, tag="sc", bufs=4)
            nc.tensor.matmul(pbc, lhsT=ones_rf, rhs=r_row, start=True, stop=True)
            rbc2 = pE.tile([128, 512], BF16, tag="rbc2")
            nc.vector.tensor_copy(rbc2, pbc)
            pbc2 = psC.tile([128, 512], F32, tag="sc", bufs=4)
            nc.tensor.matmul(pbc2, lhsT=ones_rf, rhs=mr_row, start=True, stop=True)
            mrbc = pE.tile([128, 512], BF16, tag="mrbc")
            nc.vector.tensor_copy(mrbc, pbc2)
            h2 = []
            for dk in range(DK):
                a = pE.tile([128, 512], BF16, tag=f"h2{dk}", name=f"h2{dk}")
                nc.vector.tensor_mul(a, y1bf[dk], rbc2)
                nc.vector.tensor_sub(a, a, mrbc)
                h2.append(a)
            su = []
            for m in range(NI):
                wg = pE.tile([128, DK, 128], BF16, tag="wg", bufs=3)
                nc.sync.dma_start(
                    out=wg,
                    in_=w_in_f.rearrange("(k p) m -> p k m", p=128)[:, :, m * 128:(m + 1) * 128])
                wu = pE.tile([128, DK, 128], BF16, tag="wu", bufs=3)
                nc.sync.dma_start(
                    out=wu,
                    in_=w_in_f.rearrange("(k p) m -> p k m", p=128)[:, :, FFN + m * 128:FFN + (m + 1) * 128])
                psg = psC.tile([128, 512], F32, tag="sc", bufs=4)
                for kk in range(DK):
                    nc.tensor.matmul(psg, lhsT=wg[:, kk, :],
                                     rhs=h2[kk], start=(kk == 0), stop=(kk == DK - 1))
                psu = psC.tile([128, 512], F32, tag="sc", bufs=4)
                for kk in range(DK):
                    nc.tensor.matmul(psu, lhsT=wu[:, kk, :],
                                     rhs=h2[kk], start=(kk == 0), stop=(kk == DK - 1))
                sg_t = pE.tile([128, 512], BF16, tag="sg_t", bufs=2)
                su_t = pE.tile([128, 512], BF16, tag=f"su{m}", name=f"su{m}")
                if zero_bias:
                    nc.scalar.activation(out=sg_t, in_=psg, func=Silu, scale=1.0)
                    nc.vector.tensor_mul(su_t, psu, sg_t)
                else:
                    nc.scalar.activation(out=sg_t, in_=psg, func=Silu,
                                         scale=1.0, bias=inb_t[:, m:m + 1])
                    tu = pE.tile([128, 512], F32, tag="tu", bufs=2)
                    nc.vector.tensor_scalar_add(tu, psu, inb_t[:, NI + m:NI + m + 1])
                    nc.vector.tensor_mul(su_t, tu, sg_t)
                su.append(su_t)
            # ---- w_out + final residual, straight to output ----
            for dm in range(DK):
                wot = pE.tile([128, NI, 128], BF16, tag="wot", bufs=2)
                nc.sync.dma_start(
                    out=wot,
                    in_=w_out_f.rearrange("(k p) m -> p k m", p=128)[:, :, dm * 128:(dm + 1) * 128])
                py2 = psC.tile([128, 512], F32, tag="sc", bufs=4)
                for k in range(NI):
                    nc.tensor.matmul(py2, lhsT=wot[:, k, :],
                                     rhs=su[k], start=(k == 0), stop=(k == NI - 1))
                yout = ld.tile([128, 512], BF16, tag="yout", bufs=1)
                nc.vector.tensor_add(yout, y1own[dm], py2)
                nc.sync.dma_start(out=yT_out[dm * 128:(dm + 1) * 128, :], in_=yout)
            psC_cm.__exit__(None, None, None)
            pE_cm.__exit__(None, None, None)
            pwc_cm.__exit__(None, None, None)
            pqkv_cm.__exit__(None, None, None)
            pD_cm.__exit__(None, None, None)

    nc.compile()
    return nc


def _prep_inputs(inputs):
    """Host-side shard prep: returns (lam, zero_bias, subln_c, in_maps)."""
    f = {k: np.asarray(v, dtype=np.float32) for k, v in inputs.items()}
    lam = float(np.exp(np.sum(f["lq1"] * f["lk1"]))
                - np.exp(np.sum(f["lq2"] * f["lk2"])) + LAMBDA_INIT)
    x = f["x"].reshape(NS, D)
    x_bf = x.astype(NP_BF16)
    xT = np.ascontiguousarray(x.T)                       # [D, NS]
    pt = np.arange(128)[:, None, None]
    rl = np.arange(4)[None, :, None]
    cs = np.arange(512)[None, None, :]
    masks = (pt <= cs - 128 * rl).astype(NP_BF16)
    ident = np.eye(128, dtype=NP_BF16)
    subln_base = (f["subln_w"] * (1.0 - LAMBDA_INIT)).astype(np.float32)
    s8 = float(HD) ** -0.5
    l1w = f["ln1_w"][:, None]
    wq_e = l1w * f["wq"] * s8
    wk_e = l1w * f["wk"]
    wv_e = l1w * f["wv"]
    qb_full = f["ln1_b"] @ f["wq"] * s8                  # [D]
    kb_full = f["ln1_b"] @ f["wk"]
    vb_full = f["ln1_b"] @ f["wv"]
    w_in_e = (f["ln2_w"][:, None] * f["w_in"]).astype(NP_BF16)   # [D, 2*FFN]
    inb = (f["ln2_b"] @ f["w_in"]).astype(np.float32)            # [2*FFN]
    w_out_bf = f["w_out"].astype(NP_BF16)

    zero_bias = bool(np.all(qb_full == 0.0) and np.all(kb_full == 0.0)
                     and np.all(vb_full == 0.0) and np.all(inb == 0.0))
    subln_c = None
    if np.all(subln_base == subln_base[0]):
        subln_c = float(subln_base[0])
    wo_eff = f["wo"] * (subln_c if subln_c is not None else 1.0)
    wo_bf = wo_eff.astype(NP_BF16)

    sel8_np = (np.sqrt(128.0) * np.eye(8, dtype=np.float32)[:, :, None]
               * np.ones((1, 1, 128), np.float32)).astype(NP_BF16)
    in_maps = []
    for c in range(N_CORES):
        hc = slice(128 * c, 128 * (c + 1))
        xo = np.concatenate(
            [xT[:, b * S + 512 * sl + 64 * c: b * S + 512 * sl + 64 * c + 64]
             for b in range(B) for sl in range(4)], axis=1)
        m = {
            "x_nat": x_bf,
            "sel8": sel8_np,
            "xT_own": np.ascontiguousarray(xo),
            "wq_s": wq_e[:, hc].astype(NP_BF16),
            "wk_s": wk_e[:, hc].astype(NP_BF16),
            "wv_s": wv_e[:, hc].astype(NP_BF16),
            "wo_f": wo_bf,
            "w_in_f": w_in_e,
            "w_out_f": w_out_bf,
            "masks": masks, "ident": ident,
        }
        if not zero_bias:
            m["qb"] = np.ascontiguousarray(qb_full[hc])
            m["kb"] = np.ascontiguousarray(kb_full[hc])
            m["vb4"] = np.tile(vb_full[hc], 4).astype(NP_BF16)
            m["inb"] = inb
        if subln_c is None:
            m["subln_eff"] = subln_base
        in_maps.append(m)
    return lam, zero_bias, subln_c, in_maps


_CACHE = {}


def _run(inputs, trace=False, trace_kwargs=None):
    lam, zero_bias, subln_c, in_maps = _prep_inputs(inputs)
    key = (round(lam, 10), zero_bias, subln_c)
    if key not in _CACHE:
        _CACHE[key] = build_program(lam, zero_bias, subln_c)
    nc = _CACHE[key]
    res = bass_utils.run_bass_kernel_spmd(
        nc, in_maps, core_ids=list(range(N_CORES)),
        trace=trace, **(trace_kwargs or {}))
    y = np.empty((NS, D), dtype=np.float32)
    for c in range(N_CORES):
        yT = np.asarray(res.results[c]["yT"], dtype=np.float32)  # [D, 512]
        for b in range(B):
            for sl in range(4):
                fb = b * S + 512 * sl + 64 * c
                cb = (4 * b + sl) * 64
                y[fb:fb + 64, :] = yT[:, cb:cb + 64].T
    return y.reshape(B, S, D), res


def kernel(**inputs) -> np.ndarray:
    y, _ = _run(inputs)
    return y
